# revision 1
# baseline (speedup 1.0000x reference)
"""BasesDecomposition (R-GCN style) message passing kernel for Trainium2.

Strategy (8 NeuronCores, SPMD — one program, per-core data):
  - Nodes sharded by row: core c owns targets [c*NL, (c+1)*NL).
  - Edges symmetrized on host, partitioned by target-owner core.
  - Per-relation weights W_r = sum_b rbw[r, b] * bases[b] computed on host.
  - Self-loop handled as a dense matmul with host-masked, host-transposed
    local features xm^T.
  - Phase 1 (messages): per 128-edge chunk (relation-pure, padded to a
    uniform per-relation group size G): indirect-gather x[src] rows,
    PE-transpose, matmul with W_r, write message rows sequentially to a
    DRAM buffer md. Within a relation, edges are ordered by (target
    block, rank) so md rows for one target block form one contiguous
    run per relation.
  - Phase 2 (aggregate): per 128-target block: ONE wide indirect gather
    whose 128 indices are interval starts covering the block's 32 runs
    (each index fetches SL consecutive md rows -> [128, SL*128] tile);
    for each of the SL column slices, build a one-hot*ew indicator T on
    DVE and accumulate out^T[o, t] += M_slice^T @ T on the tensor
    engine in PSUM; add the self-loop matmul W_self^T @ xm^T; store.
  - Host reassembles out from the per-core out^T blocks.
"""

import numpy as np

import concourse.bass as bass
import concourse.bacc as bacc
import concourse.tile as tile
import concourse.mybir as mybir
from concourse.bass_utils import run_bass_kernel_spmd

F32 = mybir.dt.float32
F32R = mybir.dt.float32r
I32 = mybir.dt.int32

NCORE = 8
R = 32  # num_relations (relation id R is the self-loop row of rbw)
SL_CANDIDATES = (8, 10, 12, 16)  # md rows per cover index in phase 2


def _ranks_within_group(keys, order, nbins):
    """rank of each element within its key group, following `order`."""
    counts = np.bincount(keys, minlength=nbins)
    starts = np.concatenate([[0], np.cumsum(counts)[:-1]])
    r = np.empty(len(keys), np.int64)
    r[order] = np.arange(len(keys)) - starts[keys[order]]
    return r


def host_prep(x, node_keep_mask, source, target, edge_type, edge_weights,
              bases, relation_base_weights):
    n, d = x.shape
    assert n % NCORE == 0
    nl = n // NCORE
    nblk = (nl + 127) // 128
    nlp = nblk * 128

    f32 = np.float32
    W = np.einsum("rb,bdo->rdo", relation_base_weights.astype(f32),
                  bases.astype(f32)).astype(f32)  # (R+1, 128, 128)
    wsb_h = np.ascontiguousarray(W.transpose(1, 0, 2).reshape(d, (R + 1) * d))

    src2 = np.concatenate([source, target]).astype(np.int64)
    tgt2 = np.concatenate([target, source]).astype(np.int64)
    et2 = np.concatenate([edge_type, edge_type]).astype(np.int64)
    ew2 = np.concatenate([edge_weights, edge_weights]).astype(f32)

    owner = tgt2 // nl
    tloc = tgt2 - owner * nl
    blk = tloc // 128
    tin = (tloc - blk * 128).astype(f32)

    # phase-1: uniform relation-group size G across (core, relation);
    # within a relation, order edges by target block (for phase-2 runs)
    cr = owner * R + et2
    cnt_cr = np.bincount(cr, minlength=NCORE * R)
    G = int(np.ceil(max(int(cnt_cr.max()), 1) / 128)) * 128
    ep1 = R * G
    ng1 = ep1 // 128
    order1 = np.lexsort((blk, cr))
    r1 = _ranks_within_group(cr, order1, NCORE * R)
    pos1 = et2 * G + r1  # core-local md row of each edge

    # per-(core, rel, blk) run lengths and starts (within the relation group)
    crb = cr * nblk + blk
    cnt_crb = np.bincount(crb, minlength=NCORE * R * nblk).reshape(
        NCORE, R, nblk)
    run_start = np.zeros_like(cnt_crb)
    run_start[:, :, 1:] = np.cumsum(cnt_crb, axis=2)[:, :, :-1]

    # smallest cover stride whose per-(core, block) interval count fits in
    # the 128 indices of one indirect gather
    for SL in SL_CANDIDATES:
        n_iv = np.ceil(cnt_crb / SL).sum(axis=1).max()
        if n_iv <= 128:
            break
    else:
        raise AssertionError(f"no SL fits: {n_iv} intervals")

    xf = np.ascontiguousarray(x.astype(f32))
    keep = node_keep_mask.astype(f32)

    per_core = []
    for c in range(NCORE):
        m = owner == c
        gsrc_flat = np.zeros(ep1, np.int32)
        gsrc_flat[pos1[m]] = src2[m].astype(np.int32)
        gsrc_h = np.ascontiguousarray(gsrc_flat.reshape(ng1, 128).T)

        # md row -> edge id map for this core
        edge_of_row = np.full(ep1, -1, np.int64)
        edge_ids = np.nonzero(m)[0]
        edge_of_row[pos1[edge_ids]] = edge_ids

        # phase-2 cover: per block, interval starts covering the 32 runs
        cidx_h = np.zeros((128, nblk), np.int32)
        vlen_h = np.zeros((128, nblk), np.int64)
        for b in range(nblk):
            iv = []
            for r in range(R):
                s = r * G + int(run_start[c, r, b])
                ln = int(cnt_crb[c, r, b])
                for off in range(0, ln, SL):
                    st = min(s + off, ep1 - SL)
                    iv.append((st, min(SL, s + ln - st)))
            assert len(iv) <= 128, f"cover overflow: {len(iv)} intervals"
            for p, (st, vl) in enumerate(iv):
                cidx_h[p, b] = st
                vlen_h[p, b] = vl

        # map covered rows -> (tcol, tscl) streams in cover layout
        rows = cidx_h.astype(np.int64)[:, :, None] + np.arange(SL)  # [128, nblk, SL]
        ev = edge_of_row[rows]  # [128, nblk, SL]
        in_run = np.arange(SL)[None, None, :] < vlen_h[:, :, None]
        valid = (ev >= 0) & in_run
        evc = np.where(valid, ev, 0)
        same_blk = blk[evc] == np.arange(nblk)[None, :, None]
        use = valid & same_blk
        assert int(use.sum()) == len(edge_ids), (
            f"cover mismatch: {int(use.sum())} vs {len(edge_ids)}")
        tcol_h = np.where(use, tin[evc], -1.0).astype(f32)
        tscl_h = np.where(use, ew2[evc], 0.0).astype(f32)
        tcol_h = np.ascontiguousarray(tcol_h.reshape(128, nblk * SL))
        tscl_h = np.ascontiguousarray(tscl_h.reshape(128, nblk * SL))

        xm = xf[c * nl:(c + 1) * nl] * keep[c * nl:(c + 1) * nl, None]
        xmt_h = np.zeros((128, nlp), f32)
        xmt_h[:, :nl] = xm.T

        per_core.append({
            "xg": xf,
            "wsb": wsb_h,
            "xmt": xmt_h,
            "gsrc": gsrc_h,
            "cidx": np.ascontiguousarray(cidx_h),
            "tcol": tcol_h,
            "tscl": tscl_h,
        })

    cfg = dict(n=n, nl=nl, nblk=nblk, nlp=nlp, G=G, ep1=ep1, ng1=ng1, SL=SL)
    return per_core, cfg


def build_program(cfg):
    n = cfg["n"]
    nblk = cfg["nblk"]
    nlp = cfg["nlp"]
    G = cfg["G"]
    ep1 = cfg["ep1"]
    ng1 = cfg["ng1"]
    SL = cfg["SL"]

    nc = bacc.Bacc(None, target_bir_lowering=False, debug=False)

    xg = nc.declare_dram_parameter("xg", [n, 128], F32R, isOutput=False)
    wsb = nc.declare_dram_parameter("wsb", [128, (R + 1) * 128], F32R, isOutput=False)
    xmt = nc.declare_dram_parameter("xmt", [128, nlp], F32R, isOutput=False)
    gsrc = nc.declare_dram_parameter("gsrc", [128, ng1], I32, isOutput=False)
    cidx = nc.declare_dram_parameter("cidx", [128, nblk], I32, isOutput=False)
    tcol = nc.declare_dram_parameter("tcol", [128, nblk * SL], F32, isOutput=False)
    tscl = nc.declare_dram_parameter("tscl", [128, nblk * SL], F32, isOutput=False)
    outT = nc.declare_dram_parameter("outT", [128, nlp], F32, isOutput=True)

    md = nc.dram_tensor("md", [ep1, 128], F32R)

    ident_d = nc.inline_tensor(np.eye(128, dtype=np.float32), name="ident_c")
    colidx_d = nc.inline_tensor(
        np.tile(np.arange(128, dtype=np.float32), (128, 1)), name="colidx_c")

    with tile.TileContext(nc) as tc:
        with tc.tile_pool(name="const", bufs=1) as constp:
            wsb_t = constp.tile([128, (R + 1) * 128], F32R)
            nc.sync.dma_start(out=wsb_t[:], in_=wsb[:])
            xmt_t = constp.tile([128, nlp], F32R)
            nc.sync.dma_start(out=xmt_t[:], in_=xmt[:])
            gsrc_t = constp.tile([128, ng1], I32)
            nc.sync.dma_start(out=gsrc_t[:], in_=gsrc[:])
            cidx_t = constp.tile([128, nblk], I32)
            nc.sync.dma_start(out=cidx_t[:], in_=cidx[:])
            tcol_t = constp.tile([128, nblk * SL], F32)
            nc.sync.dma_start(out=tcol_t[:], in_=tcol[:])
            tscl_t = constp.tile([128, nblk * SL], F32)
            nc.sync.dma_start(out=tscl_t[:], in_=tscl[:])
            ident_f = constp.tile([128, 128], F32)
            nc.sync.dma_start(out=ident_f[:], in_=ident_d[:])
            ident = constp.tile([128, 128], F32R)
            nc.vector.tensor_copy(out=ident[:], in_=ident_f[:])
            colidx_f = constp.tile([128, 128], F32)
            nc.sync.dma_start(out=colidx_f[:], in_=colidx_d[:])
            colidx = constp.tile([128, 128], F32R)
            nc.vector.tensor_copy(out=colidx[:], in_=colidx_f[:])

            # ---------------- Phase 1: messages ----------------
            with (
                tc.tile_pool(name="p1", bufs=20) as p1,
                tc.tile_pool(name="p1ps", bufs=3, space="PSUM") as p1ps,
            ):
                for c in range(ng1):
                    xga = p1.tile([128, 128], F32R, tag="xgather")
                    nc.gpsimd.indirect_dma_start(
                        out=xga[:], out_offset=None, in_=xg[:, :],
                        in_offset=bass.IndirectOffsetOnAxis(
                            ap=gsrc_t[:, c:c + 1], axis=0))
                    tp = p1ps.tile([128, 128], F32R, tag="tpsum")
                    nc.tensor.transpose(out=tp[:], in_=xga[:], identity=ident[:])
                    xT = p1.tile([128, 128], F32R, tag="xT")
                    nc.vector.tensor_copy(out=xT[:], in_=tp[:])
                    mp = p1ps.tile([128, 128], F32, tag="mpsum")
                    r = (c * 128) // G
                    nc.tensor.matmul(
                        out=mp[:], lhsT=xT[:],
                        rhs=wsb_t[:, 128 * r:128 * (r + 1)],
                        start=True, stop=True)
                    ms = p1.tile([128, 128], F32R, tag="mstage")
                    nc.scalar.copy(out=ms[:], in_=mp[:])
                    nc.sync.dma_start(out=md[128 * c:128 * (c + 1), :], in_=ms[:])

            # ---------------- Phase 2: aggregate ----------------
            with (
                tc.tile_pool(name="p2", bufs=10) as p2,
                tc.tile_pool(name="p2ps", bufs=4, space="PSUM") as p2ps,
            ):
                for b in range(nblk):
                    mg = p2.tile([128, SL * 128], F32R, tag="mg")
                    nc.gpsimd.indirect_dma_start(
                        out=mg[:], out_offset=None, in_=md[:, :],
                        in_offset=bass.IndirectOffsetOnAxis(
                            ap=cidx_t[:, b:b + 1], axis=0))
                    ps = p2ps.tile([128, 128], F32, tag="acc")
                    for j in range(SL):
                        tt = p2.tile([128, 128], F32R, tag="T")
                        nc.vector.tensor_scalar(
                            out=tt[:], in0=colidx[:],
                            scalar1=tcol_t[:, b * SL + j:b * SL + j + 1],
                            scalar2=tscl_t[:, b * SL + j:b * SL + j + 1],
                            op0=mybir.AluOpType.is_equal,
                            op1=mybir.AluOpType.mult)
                        nc.tensor.matmul(
                            out=ps[:],
                            lhsT=mg[:, 128 * j:128 * (j + 1)],
                            rhs=tt[:],
                            start=(j == 0), stop=False)
                    nc.tensor.matmul(
                        out=ps[:],
                        lhsT=wsb_t[:, R * 128:(R + 1) * 128],
                        rhs=xmt_t[:, 128 * b:128 * (b + 1)],
                        start=False, stop=True)
                    ob = p2.tile([128, 128], F32, tag="ob")
                    nc.vector.tensor_copy(out=ob[:], in_=ps[:])
                    nc.sync.dma_start(out=outT[:, 128 * b:128 * (b + 1)],
                                      in_=ob[:])

    nc.finalize()
    return nc


_PROGRAM_CACHE = {}


def _get_program(cfg):
    key = tuple(sorted(cfg.items()))
    if key not in _PROGRAM_CACHE:
        _PROGRAM_CACHE[key] = build_program(cfg)
    return _PROGRAM_CACHE[key]


def kernel(x, node_keep_mask, source, target, edge_type, edge_weights,
           bases, relation_base_weights):
    per_core, cfg = host_prep(x, node_keep_mask, source, target, edge_type,
                              edge_weights, bases, relation_base_weights)
    nc = _get_program(cfg)
    res = run_bass_kernel_spmd(nc, per_core, list(range(NCORE)))
    nl = cfg["nl"]
    out = np.empty((cfg["n"], 128), np.float32)
    for c in range(NCORE):
        out[c * nl:(c + 1) * nl] = res.results[c]["outT"][:, :nl].T
    return out



# revision 3
# speedup vs baseline: 3.3972x; 3.3972x over previous
"""BasesDecomposition (R-GCN style) message passing kernel for Trainium2.

V3 strategy (8 NeuronCores, SPMD — one program, per-core data):
  - Nodes sharded by row: core c owns targets [c*NL, (c+1)*NL).
  - Edges symmetrized on host, partitioned by target-owner core, then by
    half (pipeline stage over target-block ranges) and relation.
  - Host pre-gathers source features: XST[:, slot] = ew_e * x[src_e]
    (transposed, fp16, edge-weight folded in) so the device does no
    per-edge gathers and no transposes in phase 1.
  - Phase 1 (messages): per 128-edge relation-pure chunk, one fp16
    matmul XST_chunk.T @ W_r -> PSUM; NG chunks share one PSUM bank,
    one fp32->fp16 copy, and one grouped DMA write to the message
    buffer md. md slot map row = rowbase_g + e*n_g + j keeps the write
    contiguous per partition while ranks (target-block-sorted order
    within each relation) stay contiguous for phase-2 interval gathers.
  - Phase 2 (aggregate): per 128-target block, ONE indirect interval
    gather (128 intervals x SL rows) covering the block's per-relation
    runs; ONE broadcast is_equal builds all SL one-hot matrices; SL
    scatter matmuls accumulate out[t, o] += T_j.T @ M_j in PSUM, plus a
    self-loop matmul; direct DMA to the output.
  - Two halves are pipelined: phase 2 of half 0 is interleaved with
    phase 1 of half 1 so DMA/gpsimd/PE work overlaps across halves.
"""

import numpy as np

import concourse.bass as bass
import concourse.bacc as bacc
import concourse.tile as tile
import concourse.mybir as mybir
from concourse.bass_utils import run_bass_kernel_spmd

F16 = mybir.dt.float16
F32 = mybir.dt.float32
I32 = mybir.dt.int32

NCORE = 8
H = 2            # pipeline halves (target-block ranges)
NG = 4           # chunks per PSUM group / md write
KR = 8           # chunks per XST read DMA
SLC = (8, 10, 12, 16)  # md rows per cover interval in phase 2


def _ranks_within_group(keys, order, nbins):
    counts = np.bincount(keys, minlength=nbins)
    starts = np.concatenate([[0], np.cumsum(counts)[:-1]])
    r = np.empty(len(keys), np.int64)
    r[order] = np.arange(len(keys)) - starts[keys[order]]
    return r


def host_prep(x, node_keep_mask, source, target, edge_type, edge_weights,
              bases, relation_base_weights):
    n, d = x.shape
    assert d == 128 and n % NCORE == 0
    R = relation_base_weights.shape[0] - 1
    nl = n // NCORE
    nblk = (nl + 127) // 128
    nlp = nblk * 128
    nbA = (nblk + 1) // 2
    hb = [(0, nbA), (nbA, nblk)]
    f16, f32 = np.float16, np.float32

    W = np.einsum("rb,bdo->rdo", relation_base_weights.astype(f32),
                  bases.astype(f32)).astype(f32)
    wsb_h = np.ascontiguousarray(
        W.transpose(1, 0, 2).reshape(d, (R + 1) * d)).astype(f16)

    x16 = x.astype(f16).astype(f32)  # quantize once, scale in f32
    src2 = np.concatenate([source, target]).astype(np.int64)
    tgt2 = np.concatenate([target, source]).astype(np.int64)
    et2 = np.concatenate([edge_type, edge_type]).astype(np.int64)
    ew2 = np.concatenate([edge_weights, edge_weights]).astype(f32)

    owner = tgt2 // nl
    tloc = tgt2 - owner * nl
    blkg = tloc // 128
    tin = (tloc - blkg * 128).astype(f32)
    halfid = (blkg >= nbA).astype(np.int64)

    # shared (cross-core) chunk structure from per-(core, half, rel) maxima
    cnt_chr = np.bincount(
        (owner * H + halfid) * R + et2, minlength=NCORE * H * R
    ).reshape(NCORE, H, R)
    SLMAX = max(SLC)
    nch_hr = np.ceil((cnt_chr.max(axis=0) + SLMAX) / 128.0).astype(np.int64)

    cnt_chrb = np.bincount(
        ((owner * H + halfid) * R + et2) * nblk + blkg,
        minlength=NCORE * H * R * nblk).reshape(NCORE, H, R, nblk)

    halves = []
    for h in range(H):
        b0, b1 = hb[h]
        nch_r = nch_hr[h]
        cb = np.concatenate([[0], np.cumsum(nch_r)]).astype(np.int64)
        nch_h = int(cb[-1])
        groups = []
        for r in range(R):
            for g0 in range(0, int(nch_r[r]), NG):
                ng_ = int(min(NG, int(nch_r[r]) - g0))
                cf = int(cb[r] + g0)
                groups.append((128 * cf, cf, ng_, r))
        for SL in SLC:
            n_iv = int(np.ceil(cnt_chrb[:, h, :, b0:b1] / SL)
                       .sum(axis=1).max())
            if n_iv <= 128:
                break
        else:
            raise AssertionError(f"no SL fits: {n_iv}")
        halves.append(dict(nch=nch_h, ep1=128 * nch_h, cb=cb, nch_r=nch_r,
                           groups=tuple(groups), b0=b0, b1=b1,
                           nbh=b1 - b0, SL=int(SL)))

    per_core = []
    for c in range(NCORE):
        dcore = {"wsb": wsb_h}
        xm = (x16[c * nl:(c + 1) * nl]
              * node_keep_mask[c * nl:(c + 1) * nl, None])
        xmt = np.zeros((128, nlp), f16)
        xmt[:, :nl] = xm.T.astype(f16)
        dcore["xmt"] = np.ascontiguousarray(xmt)
        for h in range(H):
            hs = halves[h]
            b0, nbh, SL = hs["b0"], hs["nbh"], hs["SL"]
            cbs, nch_r, nch_h, ep1 = hs["cb"], hs["nch_r"], hs["nch"], hs["ep1"]
            sel = np.nonzero((owner == c) & (halfid == h))[0]
            et_s = et2[sel]
            blk_s = blkg[sel] - b0
            order = np.lexsort((blk_s, et_s))
            ranks = _ranks_within_group(et_s, order, R)
            rows = 128 * cbs[et_s] + ranks
            nch_of = nch_r[et_s]
            g = ranks // (128 * NG)
            ng_of = np.minimum(NG, nch_of - NG * g)
            om = ranks - g * 128 * NG
            e = om // ng_of
            jj = om - e * ng_of
            chunk = cbs[et_s] + NG * g + jj
            xcol = chunk * 128 + e
            XS = np.zeros((128 * nch_h, d), f16)
            XS[xcol] = (x16[src2[sel]] * ew2[sel][:, None]).astype(f16)
            dcore[f"xst{h}"] = np.ascontiguousarray(XS.T)

            edge_of_row = np.full(ep1, -1, np.int64)
            edge_of_row[rows] = sel
            cnt_rb = cnt_chrb[c, h, :, b0:hs["b1"]]
            run_start = np.zeros_like(cnt_rb)
            run_start[:, 1:] = np.cumsum(cnt_rb, axis=1)[:, :-1]
            # pad entries point at the tail pad rows (written, no edges)
            cidx = np.full((128, nbh), ep1 - SL, np.int32)
            for b in range(nbh):
                iv = []
                for r in range(R):
                    s = 128 * int(cbs[r]) + int(run_start[r, b])
                    ln = int(cnt_rb[r, b])
                    limit = 128 * int(cbs[r] + nch_r[r]) - SL
                    for off in range(0, ln, SL):
                        iv.append(min(s + off, limit))
                assert len(iv) <= 128, f"cover overflow {len(iv)}"
                cidx[:len(iv), b] = iv
            rowsm = cidx.astype(np.int64)[:, :, None] + np.arange(SL)
            evm = edge_of_row[rowsm]
            valid = evm >= 0
            evc = np.where(valid, evm, 0)
            use = (valid
                   & ((blkg[evc] - b0) == np.arange(nbh)[None, :, None])
                   & (halfid[evc] == h))
            assert int(use.sum()) == len(sel), (
                f"cover mismatch {int(use.sum())} vs {len(sel)}")
            tcol = np.where(use, tin[evc], -1.0).astype(f16)
            dcore[f"cidx{h}"] = np.ascontiguousarray(cidx)
            dcore[f"tcol{h}"] = np.ascontiguousarray(
                tcol.reshape(128, nbh * SL))
        per_core.append(dcore)

    cfg = dict(R=R, nlp=nlp, nblk=nblk,
               halves=tuple((hs["nch"], hs["ep1"], hs["b0"], hs["b1"],
                             hs["SL"], hs["groups"]) for hs in halves))
    return per_core, cfg


def build_program(cfg):
    R = cfg["R"]
    nlp = cfg["nlp"]
    SLmax = max(hv[4] for hv in cfg["halves"])

    nc = bacc.Bacc(None, target_bir_lowering=False, debug=False)
    wsb = nc.declare_dram_parameter("wsb", [128, (R + 1) * 128], F16,
                                    isOutput=False)
    xmt = nc.declare_dram_parameter("xmt", [128, nlp], F16, isOutput=False)
    hp = []
    for h, (nch_h, ep1, b0, b1, SL, groups) in enumerate(cfg["halves"]):
        xst = nc.declare_dram_parameter(f"xst{h}", [128, nch_h * 128], F16,
                                        isOutput=False)
        cidx = nc.declare_dram_parameter(f"cidx{h}", [128, b1 - b0], I32,
                                         isOutput=False)
        tcol = nc.declare_dram_parameter(f"tcol{h}", [128, (b1 - b0) * SL],
                                         F16, isOutput=False)
        md = nc.dram_tensor(f"md{h}", [ep1, 128], F16)
        hp.append((xst, cidx, tcol, md))
    outp = nc.declare_dram_parameter("out", [nlp, 128], F32, isOutput=True)

    colidx_d = nc.inline_tensor(
        np.tile(np.arange(128, dtype=np.float16), (128, SLmax)),
        name="colidx_c")

    with tile.TileContext(nc) as tc:
        with (
            tc.tile_pool(name="const", bufs=1) as constp,
            tc.tile_pool(name="rd", bufs=6) as rdp,
            tc.tile_pool(name="msb", bufs=6) as msbp,
            tc.tile_pool(name="p1ps", bufs=4, space="PSUM") as p1ps,
            tc.tile_pool(name="mg", bufs=6) as mgp,
            tc.tile_pool(name="tt", bufs=6) as ttp,
            tc.tile_pool(name="ob", bufs=4) as obp,
            tc.tile_pool(name="p2ps", bufs=4, space="PSUM") as p2ps,
        ):
            wsb_t = constp.tile([128, (R + 1) * 128], F16)
            nc.sync.dma_start(out=wsb_t[:], in_=wsb[:])
            xmt_t = constp.tile([128, nlp], F16)
            nc.sync.dma_start(out=xmt_t[:], in_=xmt[:])
            colidx = constp.tile([128, SLmax, 128], F16)
            nc.sync.dma_start(out=colidx[:], in_=colidx_d[:])
            cidx_ts, tcol_ts = [], []
            for h, (nch_h, ep1, b0, b1, SL, groups) in enumerate(
                    cfg["halves"]):
                nbh = b1 - b0
                ct = constp.tile([128, nbh], I32, name=f"cidx_t{h}")
                nc.sync.dma_start(out=ct[:], in_=hp[h][1][:])
                tc_ = constp.tile([128, nbh * SL], F16, name=f"tcol_t{h}")
                nc.sync.dma_start(out=tc_[:], in_=hp[h][2][:])
                cidx_ts.append(ct)
                tcol_ts.append(tc_)

            read_cache = [dict() for _ in range(H)]

            def get_read(h, ci):
                nch_h = cfg["halves"][h][0]
                bi = ci // KR
                rc = read_cache[h]
                if bi not in rc:
                    w = min(KR, nch_h - bi * KR)
                    rt = rdp.tile([128, KR * 128], F16, tag="rt")
                    nc.sync.dma_start(
                        out=rt[:, :w * 128],
                        in_=hp[h][0][:, bi * KR * 128:(bi * KR + w) * 128])
                    rc.clear()
                    rc[bi] = rt
                return rc[bi]

            def emit_p1_group(h, gi):
                groups = cfg["halves"][h][5]
                md_d = hp[h][3]
                rowbase, cf, ng_, rel = groups[gi]
                mp = p1ps.tile([128, NG * 128], F32, tag="mp")
                for j in range(ng_):
                    ci = cf + j
                    rt = get_read(h, ci)
                    off = (ci % KR) * 128
                    nc.tensor.matmul(
                        out=mp[:, j * 128:(j + 1) * 128],
                        lhsT=rt[:, off:off + 128],
                        rhs=wsb_t[:, rel * 128:(rel + 1) * 128],
                        start=True, stop=True)
                msb = msbp.tile([128, NG * 128], F16, tag="msb")
                nc.vector.tensor_copy(out=msb[:, :ng_ * 128],
                                      in_=mp[:, :ng_ * 128])
                dst = md_d[rowbase:rowbase + 128 * ng_, :].rearrange(
                    "(e j) o -> e j o", j=ng_)
                nc.scalar.dma_start(out=dst, in_=msb[:, :ng_ * 128])

            def emit_p2_block(h, b):
                nch_h, ep1, b0, b1, SL, groups = cfg["halves"][h]
                md_d = hp[h][3]
                mg = mgp.tile([128, SLmax * 128], F16, tag="mg")
                nc.gpsimd.indirect_dma_start(
                    out=mg[:, :SL * 128], out_offset=None, in_=md_d[:, :],
                    in_offset=bass.IndirectOffsetOnAxis(
                        ap=cidx_ts[h][:, b:b + 1], axis=0))
                tt = ttp.tile([128, SLmax, 128], F16, tag="tt")
                nc.vector.tensor_tensor(
                    out=tt[:, :SL, :], in0=colidx[:, :SL, :],
                    in1=tcol_ts[h][:, b * SL:(b + 1) * SL].unsqueeze(2)
                    .to_broadcast([128, SL, 128]),
                    op=mybir.AluOpType.is_equal)
                ps = p2ps.tile([128, 128], F32, tag="acc")
                for j in range(SL):
                    nc.tensor.matmul(
                        out=ps[:], lhsT=tt[:, j, :],
                        rhs=mg[:, j * 128:(j + 1) * 128],
                        start=(j == 0), stop=False)
                gb = b0 + b
                nc.tensor.matmul(
                    out=ps[:], lhsT=xmt_t[:, gb * 128:(gb + 1) * 128],
                    rhs=wsb_t[:, R * 128:(R + 1) * 128],
                    start=False, stop=True)
                ob = obp.tile([128, 128], F32, tag="ob")
                nc.scalar.copy(out=ob[:], in_=ps[:])
                nc.sync.dma_start(out=outp[gb * 128:(gb + 1) * 128, :],
                                  in_=ob[:])

            # schedule: p1(0); then p2(0) interleaved with p1(1); then p2(1)
            ng0 = len(cfg["halves"][0][5])
            ng1 = len(cfg["halves"][1][5])
            nb0 = cfg["halves"][0][3] - cfg["halves"][0][2]
            nb1 = cfg["halves"][1][3] - cfg["halves"][1][2]
            for gi in range(ng0):
                emit_p1_group(0, gi)
            k = 0
            for b in range(nb0):
                emit_p2_block(0, b)
                take = ((b + 1) * ng1) // nb0 - (b * ng1) // nb0
                for _ in range(take):
                    emit_p1_group(1, k)
                    k += 1
            while k < ng1:
                emit_p1_group(1, k)
                k += 1
            for b in range(nb1):
                emit_p2_block(1, b)

    nc.finalize()
    return nc


_PROGRAM_CACHE = {}


def _get_program(cfg):
    key = (cfg["R"], cfg["nlp"], cfg["nblk"], cfg["halves"])
    if key not in _PROGRAM_CACHE:
        _PROGRAM_CACHE[key] = build_program(cfg)
    return _PROGRAM_CACHE[key]


def kernel(x, node_keep_mask, source, target, edge_type, edge_weights,
           bases, relation_base_weights):
    per_core, cfg = host_prep(x, node_keep_mask, source, target, edge_type,
                              edge_weights, bases, relation_base_weights)
    nc = _get_program(cfg)
    res = run_bass_kernel_spmd(nc, per_core, list(range(NCORE)))
    n = x.shape[0]
    nl = n // NCORE
    out = np.empty((n, 128), np.float32)
    for c in range(NCORE):
        out[c * nl:(c + 1) * nl] = res.results[c]["out"][:nl]
    return out


# revision 6
# speedup vs baseline: 3.5043x; 1.0315x over previous
"""BasesDecomposition (R-GCN style) message passing kernel for Trainium2.

V4 strategy (8 NeuronCores, SPMD — one program, per-core data):
  - Nodes sharded by row: core c owns targets [c*NL, (c+1)*NL).
  - Edges symmetrized on host, partitioned by target-owner core, then by
    pipeline third (target-block range) and relation.
  - Host pre-gathers source features: XST[:, slot] = ew_e * x[src_e]
    (transposed, fp16, edge-weight folded in) so the device does no
    per-edge gathers and no transposes in phase 1.
  - Phase 1 (messages): per 128-edge relation-pure chunk, one fp16
    matmul XST_chunk.T @ W_r -> PSUM (4 chunks per PSUM bank, one
    fp32->fp16 copy each, alternating vector/scalar engines). Up to WG
    chunks share ONE grouped DMA write to the message buffer md. The md
    slot map row = rowbase_g + e*n_g + j keeps the write contiguous per
    partition while ranks (target-block-sorted within each relation)
    stay contiguous for phase-2 interval gathers.
  - Phase 2 (aggregate): per 128-target block, ONE indirect interval
    gather (128 intervals x SL rows) covering the block's per-relation
    runs; ONE broadcast is_equal builds all SL one-hot matrices; SL
    scatter matmuls accumulate out[t, o] += T_j.T @ M_j in PSUM, plus a
    self-loop matmul; fp16 copy + direct DMA to the output.
  - H pipeline stages: phase 2 of stage h-1 is interleaved with phase 1
    of stage h so DMA/gpsimd/PE/DVE work overlaps across stages.
"""

import numpy as np

import concourse.bass as bass
import concourse.bacc as bacc
import concourse.tile as tile
import concourse.mybir as mybir
from concourse.bass_utils import run_bass_kernel_spmd

F16 = mybir.dt.float16
F32 = mybir.dt.float32
I32 = mybir.dt.int32

NCORE = 8
H = 3            # pipeline stages (target-block ranges)
WG = 16          # chunks per md write group (slot-map group size)
PG = 4           # chunks per PSUM bank / cast
KR = 8           # chunks per XST read DMA
SLC = (8, 10, 12, 16)  # md rows per cover interval in phase 2


def _ranks_within_group(keys, order, nbins):
    counts = np.bincount(keys, minlength=nbins)
    starts = np.concatenate([[0], np.cumsum(counts)[:-1]])
    r = np.empty(len(keys), np.int64)
    r[order] = np.arange(len(keys)) - starts[keys[order]]
    return r


def host_prep(x, node_keep_mask, source, target, edge_type, edge_weights,
              bases, relation_base_weights):
    n, d = x.shape
    assert d == 128 and n % NCORE == 0
    R = relation_base_weights.shape[0] - 1
    nl = n // NCORE
    nblk = (nl + 127) // 128
    nlp = nblk * 128
    # H near-even block ranges
    bnds = [round(nblk * i / H) for i in range(H + 1)]
    hb = [(bnds[i], bnds[i + 1]) for i in range(H)]
    f16, f32 = np.float16, np.float32

    W = np.einsum("rb,bdo->rdo", relation_base_weights.astype(f32),
                  bases.astype(f32)).astype(f32)
    wsb_h = np.ascontiguousarray(
        W.transpose(1, 0, 2).reshape(d, (R + 1) * d)).astype(f16)

    x16 = x.astype(f16).astype(f32)  # quantize once, scale in f32
    src2 = np.concatenate([source, target]).astype(np.int64)
    tgt2 = np.concatenate([target, source]).astype(np.int64)
    et2 = np.concatenate([edge_type, edge_type]).astype(np.int64)
    ew2 = np.concatenate([edge_weights, edge_weights]).astype(f32)

    owner = tgt2 // nl
    tloc = tgt2 - owner * nl
    blkg = tloc // 128
    tin = (tloc - blkg * 128).astype(f32)
    halfid = np.digitize(blkg, bnds[1:-1])

    cnt_chr = np.bincount(
        (owner * H + halfid) * R + et2, minlength=NCORE * H * R
    ).reshape(NCORE, H, R)
    SLMAX = max(SLC)
    nch_hr = np.ceil((cnt_chr.max(axis=0) + SLMAX) / 128.0).astype(np.int64)

    cnt_chrb = np.bincount(
        ((owner * H + halfid) * R + et2) * nblk + blkg,
        minlength=NCORE * H * R * nblk).reshape(NCORE, H, R, nblk)

    halves = []
    for h in range(H):
        b0, b1 = hb[h]
        nch_r = nch_hr[h]
        cb = np.concatenate([[0], np.cumsum(nch_r)]).astype(np.int64)
        nch_h = int(cb[-1])
        groups = []
        for r in range(R):
            for g0 in range(0, int(nch_r[r]), WG):
                ng_ = int(min(WG, int(nch_r[r]) - g0))
                cf = int(cb[r] + g0)
                groups.append((128 * cf, cf, ng_, r))
        for SL in SLC:
            n_iv = int(np.ceil(cnt_chrb[:, h, :, b0:b1] / SL)
                       .sum(axis=1).max())
            if n_iv <= 128:
                break
        else:
            raise AssertionError(f"no SL fits: {n_iv}")
        halves.append(dict(nch=nch_h, ep1=128 * nch_h, cb=cb, nch_r=nch_r,
                           groups=tuple(groups), b0=b0, b1=b1,
                           nbh=b1 - b0, SL=int(SL)))

    per_core = []
    for c in range(NCORE):
        dcore = {"wsb": wsb_h}
        xm = (x16[c * nl:(c + 1) * nl]
              * node_keep_mask[c * nl:(c + 1) * nl, None])
        xmt = np.zeros((128, nlp), f16)
        xmt[:, :nl] = xm.T.astype(f16)
        dcore["xmt"] = np.ascontiguousarray(xmt)
        for h in range(H):
            hs = halves[h]
            b0, nbh, SL = hs["b0"], hs["nbh"], hs["SL"]
            cbs, nch_r, nch_h, ep1 = (hs["cb"], hs["nch_r"], hs["nch"],
                                      hs["ep1"])
            sel = np.nonzero((owner == c) & (halfid == h))[0]
            et_s = et2[sel]
            blk_s = blkg[sel] - b0
            order = np.lexsort((blk_s, et_s))
            ranks = _ranks_within_group(et_s, order, R)
            rows = 128 * cbs[et_s] + ranks
            nch_of = nch_r[et_s]
            g = ranks // (128 * WG)
            ng_of = np.minimum(WG, nch_of - WG * g)
            om = ranks - g * 128 * WG
            e = om // ng_of
            jj = om - e * ng_of
            chunk = cbs[et_s] + WG * g + jj
            xcol = chunk * 128 + e
            XS = np.zeros((128 * nch_h, d), f16)
            XS[xcol] = (x16[src2[sel]] * ew2[sel][:, None]).astype(f16)
            dcore[f"xst{h}"] = np.ascontiguousarray(XS.T)

            edge_of_row = np.full(ep1, -1, np.int64)
            edge_of_row[rows] = sel
            cnt_rb = cnt_chrb[c, h, :, b0:hs["b1"]]
            run_start = np.zeros_like(cnt_rb)
            run_start[:, 1:] = np.cumsum(cnt_rb, axis=1)[:, :-1]
            # pad entries point at the tail pad rows (written, no edges)
            cidx = np.full((128, nbh), ep1 - SL, np.int32)
            for b in range(nbh):
                iv = []
                for r in range(R):
                    s = 128 * int(cbs[r]) + int(run_start[r, b])
                    ln = int(cnt_rb[r, b])
                    limit = 128 * int(cbs[r] + nch_r[r]) - SL
                    for off in range(0, ln, SL):
                        iv.append(min(s + off, limit))
                assert len(iv) <= 128, f"cover overflow {len(iv)}"
                cidx[:len(iv), b] = iv
            rowsm = cidx.astype(np.int64)[:, :, None] + np.arange(SL)
            evm = edge_of_row[rowsm]
            valid = evm >= 0
            evc = np.where(valid, evm, 0)
            use = (valid
                   & ((blkg[evc] - b0) == np.arange(nbh)[None, :, None])
                   & (halfid[evc] == h))
            assert int(use.sum()) == len(sel), (
                f"cover mismatch {int(use.sum())} vs {len(sel)}")
            tcol = np.where(use, tin[evc], -1.0).astype(f16)
            dcore[f"cidx{h}"] = np.ascontiguousarray(cidx)
            dcore[f"tcol{h}"] = np.ascontiguousarray(
                tcol.reshape(128, nbh * SL))
        per_core.append(dcore)

    cfg = dict(R=R, nlp=nlp, nblk=nblk,
               halves=tuple((hs["nch"], hs["ep1"], hs["b0"], hs["b1"],
                             hs["SL"], hs["groups"]) for hs in halves))
    return per_core, cfg


def build_program(cfg):
    R = cfg["R"]
    nlp = cfg["nlp"]
    SLmax = max(hv[4] for hv in cfg["halves"])

    nc = bacc.Bacc(None, target_bir_lowering=False, debug=False)
    wsb = nc.declare_dram_parameter("wsb", [128, (R + 1) * 128], F16,
                                    isOutput=False)
    xmt = nc.declare_dram_parameter("xmt", [128, nlp], F16, isOutput=False)
    hp = []
    for h, (nch_h, ep1, b0, b1, SL, groups) in enumerate(cfg["halves"]):
        xst = nc.declare_dram_parameter(f"xst{h}", [128, nch_h * 128], F16,
                                        isOutput=False)
        cidx = nc.declare_dram_parameter(f"cidx{h}", [128, b1 - b0], I32,
                                         isOutput=False)
        tcol = nc.declare_dram_parameter(f"tcol{h}", [128, (b1 - b0) * SL],
                                         F16, isOutput=False)
        md = nc.dram_tensor(f"md{h}", [ep1, 128], F16)
        hp.append((xst, cidx, tcol, md))
    outp = nc.declare_dram_parameter("out", [nlp, 128], F16, isOutput=True)

    colidx_d = nc.inline_tensor(
        np.tile(np.arange(128, dtype=np.float16), (128, SLmax)),
        name="colidx_c")

    with tile.TileContext(nc) as tc:
        with (
            tc.tile_pool(name="const", bufs=1) as constp,
            tc.tile_pool(name="rd", bufs=8) as rdp,
            tc.tile_pool(name="msb", bufs=4) as msbp,
            tc.tile_pool(name="p1ps", bufs=4, space="PSUM") as p1ps,
            tc.tile_pool(name="mg", bufs=6) as mgp,
            tc.tile_pool(name="tt", bufs=6) as ttp,
            tc.tile_pool(name="ob", bufs=6) as obp,
            tc.tile_pool(name="p2ps", bufs=4, space="PSUM") as p2ps,
        ):
            wsb_t = constp.tile([128, (R + 1) * 128], F16)
            nc.sync.dma_start(out=wsb_t[:], in_=wsb[:])
            xmt_t = constp.tile([128, nlp], F16)
            nc.sync.dma_start(out=xmt_t[:], in_=xmt[:])
            colidx = constp.tile([128, SLmax, 128], F16)
            nc.sync.dma_start(out=colidx[:], in_=colidx_d[:])
            cidx_ts, tcol_ts = [], []
            for h, (nch_h, ep1, b0, b1, SL, groups) in enumerate(
                    cfg["halves"]):
                nbh = b1 - b0
                ct = constp.tile([128, nbh], I32, name=f"cidx_t{h}")
                nc.sync.dma_start(out=ct[:], in_=hp[h][1][:])
                tc_ = constp.tile([128, nbh * SL], F16, name=f"tcol_t{h}")
                nc.sync.dma_start(out=tc_[:], in_=hp[h][2][:])
                cidx_ts.append(ct)
                tcol_ts.append(tc_)

            read_cache = [dict() for _ in range(H)]
            alt = [0]  # cast engine alternator

            def get_read(h, ci):
                nch_h = cfg["halves"][h][0]
                bi = ci // KR
                rc = read_cache[h]
                if bi not in rc:
                    w = min(KR, nch_h - bi * KR)
                    rt = rdp.tile([128, KR * 128], F16, tag="rt")
                    nc.sync.dma_start(
                        out=rt[:, :w * 128],
                        in_=hp[h][0][:, bi * KR * 128:(bi * KR + w) * 128])
                    rc.clear()
                    rc[bi] = rt
                return rc[bi]

            def emit_p1_group(h, gi):
                groups = cfg["halves"][h][5]
                md_d = hp[h][3]
                rowbase, cf, ng_, rel = groups[gi]
                msb = msbp.tile([128, WG * 128], F16, tag="msb")
                for s0 in range(0, ng_, PG):
                    sn = min(PG, ng_ - s0)
                    mp = p1ps.tile([128, PG * 128], F32, tag="mp")
                    for j in range(sn):
                        ci = cf + s0 + j
                        rt = get_read(h, ci)
                        off = (ci % KR) * 128
                        nc.tensor.matmul(
                            out=mp[:, j * 128:(j + 1) * 128],
                            lhsT=rt[:, off:off + 128],
                            rhs=wsb_t[:, rel * 128:(rel + 1) * 128],
                            start=True, stop=True)
                    if alt[0] % 2 == 0:
                        nc.vector.tensor_copy(
                            out=msb[:, s0 * 128:(s0 + sn) * 128],
                            in_=mp[:, :sn * 128])
                    else:
                        nc.scalar.copy(
                            out=msb[:, s0 * 128:(s0 + sn) * 128],
                            in_=mp[:, :sn * 128])
                    alt[0] += 1
                dst = md_d[rowbase:rowbase + 128 * ng_, :].rearrange(
                    "(e j) o -> e j o", j=ng_)
                nc.scalar.dma_start(out=dst, in_=msb[:, :ng_ * 128])

            def emit_p2_block(h, b):
                nch_h, ep1, b0, b1, SL, groups = cfg["halves"][h]
                md_d = hp[h][3]
                mg = mgp.tile([128, SLmax * 128], F16, tag="mg")
                nc.gpsimd.indirect_dma_start(
                    out=mg[:, :SL * 128], out_offset=None, in_=md_d[:, :],
                    in_offset=bass.IndirectOffsetOnAxis(
                        ap=cidx_ts[h][:, b:b + 1], axis=0))
                tt = ttp.tile([128, SLmax, 128], F16, tag="tt")
                nc.vector.tensor_tensor(
                    out=tt[:, :SL, :], in0=colidx[:, :SL, :],
                    in1=tcol_ts[h][:, b * SL:(b + 1) * SL].unsqueeze(2)
                    .to_broadcast([128, SL, 128]),
                    op=mybir.AluOpType.is_equal)
                ps = p2ps.tile([128, 128], F32, tag="acc")
                for j in range(SL):
                    nc.tensor.matmul(
                        out=ps[:], lhsT=tt[:, j, :],
                        rhs=mg[:, j * 128:(j + 1) * 128],
                        start=(j == 0), stop=False)
                gb = b0 + b
                nc.tensor.matmul(
                    out=ps[:], lhsT=xmt_t[:, gb * 128:(gb + 1) * 128],
                    rhs=wsb_t[:, R * 128:(R + 1) * 128],
                    start=False, stop=True)
                ob = obp.tile([128, 128], F16, tag="ob")
                nc.scalar.copy(out=ob[:], in_=ps[:])
                nc.sync.dma_start(out=outp[gb * 128:(gb + 1) * 128, :],
                                  in_=ob[:])

            # schedule: p1(0); for h>=1: p2(h-1) interleaved with p1(h);
            # then p2(H-1)
            ngs = [len(cfg["halves"][h][5]) for h in range(H)]
            nbs = [cfg["halves"][h][3] - cfg["halves"][h][2]
                   for h in range(H)]
            for gi in range(ngs[0]):
                emit_p1_group(0, gi)
            for h in range(1, H):
                k = 0
                for b in range(nbs[h - 1]):
                    emit_p2_block(h - 1, b)
                    take = ((b + 1) * ngs[h]) // nbs[h - 1] \
                        - (b * ngs[h]) // nbs[h - 1]
                    for _ in range(take):
                        emit_p1_group(h, k)
                        k += 1
                while k < ngs[h]:
                    emit_p1_group(h, k)
                    k += 1
            for b in range(nbs[H - 1]):
                emit_p2_block(H - 1, b)

    nc.finalize()
    return nc


_PROGRAM_CACHE = {}


def _get_program(cfg):
    key = (cfg["R"], cfg["nlp"], cfg["nblk"], cfg["halves"])
    if key not in _PROGRAM_CACHE:
        _PROGRAM_CACHE[key] = build_program(cfg)
    return _PROGRAM_CACHE[key]


def kernel(x, node_keep_mask, source, target, edge_type, edge_weights,
           bases, relation_base_weights):
    per_core, cfg = host_prep(x, node_keep_mask, source, target, edge_type,
                              edge_weights, bases, relation_base_weights)
    nc = _get_program(cfg)
    res = run_bass_kernel_spmd(nc, per_core, list(range(NCORE)))
    n = x.shape[0]
    nl = n // NCORE
    out = np.empty((n, 128), np.float32)
    for c in range(NCORE):
        out[c * nl:(c + 1) * nl] = res.results[c]["out"][:nl].astype(
            np.float32)
    return out


# revision 16
# speedup vs baseline: 3.5222x; 1.0051x over previous
"""BasesDecomposition (R-GCN style) message passing kernel for Trainium2.

V4 strategy (8 NeuronCores, SPMD — one program, per-core data):
  - Nodes sharded by row: core c owns targets [c*NL, (c+1)*NL).
  - Edges symmetrized on host, partitioned by target-owner core, then by
    pipeline third (target-block range) and relation.
  - Host pre-gathers source features: XST[:, slot] = ew_e * x[src_e]
    (transposed, fp16, edge-weight folded in) so the device does no
    per-edge gathers and no transposes in phase 1.
  - Phase 1 (messages): per 128-edge relation-pure chunk, one fp16
    matmul XST_chunk.T @ W_r -> PSUM (4 chunks per PSUM bank, one
    fp32->fp16 copy each, alternating vector/scalar engines). Up to WG
    chunks share ONE grouped DMA write to the message buffer md. The md
    slot map row = rowbase_g + e*n_g + j keeps the write contiguous per
    partition while ranks (target-block-sorted within each relation)
    stay contiguous for phase-2 interval gathers.
  - Phase 2 (aggregate): per 128-target block, ONE indirect interval
    gather (128 intervals x SL rows) covering the block's per-relation
    runs; ONE broadcast is_equal builds all SL one-hot matrices; SL
    scatter matmuls accumulate out[t, o] += T_j.T @ M_j in PSUM, plus a
    self-loop matmul; fp16 copy + direct DMA to the output.
  - H pipeline stages: phase 2 of stage h-1 is interleaved with phase 1
    of stage h so DMA/gpsimd/PE/DVE work overlaps across stages.
"""

import numpy as np

import concourse.bass as bass
import concourse.bacc as bacc
import concourse.tile as tile
import concourse.mybir as mybir
from concourse.bass_utils import run_bass_kernel_spmd

F16 = mybir.dt.float16
F32 = mybir.dt.float32
I32 = mybir.dt.int32

NCORE = 8
H = 4            # pipeline stages (target-block ranges)
HW_ = (0.2, 0.3, 0.3, 0.2)  # stage size fractions (taper head/tail)
WG = 16          # chunks per md write group (slot-map group size)
PG = 4           # chunks per PSUM bank / cast
KR = 8           # chunks per XST read DMA
OB = 4           # output blocks per batched write
SLC = (8, 10, 12, 16)  # md rows per cover interval in phase 2


def _ranks_within_group(keys, order, nbins):
    counts = np.bincount(keys, minlength=nbins)
    starts = np.concatenate([[0], np.cumsum(counts)[:-1]])
    r = np.empty(len(keys), np.int64)
    r[order] = np.arange(len(keys)) - starts[keys[order]]
    return r


def host_prep(x, node_keep_mask, source, target, edge_type, edge_weights,
              bases, relation_base_weights):
    n, d = x.shape
    assert d == 128 and n % NCORE == 0
    R = relation_base_weights.shape[0] - 1
    nl = n // NCORE
    nblk = (nl + 127) // 128
    nlp = nblk * 128
    # H block ranges, tapered so first/last stages are smaller
    cw = np.cumsum((0.0,) + HW_) / sum(HW_)
    bnds = [round(nblk * float(c)) for c in cw]
    hb = [(bnds[i], bnds[i + 1]) for i in range(H)]
    f16, f32 = np.float16, np.float32

    W = np.einsum("rb,bdo->rdo", relation_base_weights.astype(f32),
                  bases.astype(f32)).astype(f32)
    wsb_h = np.ascontiguousarray(
        W.transpose(1, 0, 2).reshape(d, (R + 1) * d)).astype(f16)

    x16 = x.astype(f16).astype(f32)  # quantize once, scale in f32
    src2 = np.concatenate([source, target]).astype(np.int64)
    tgt2 = np.concatenate([target, source]).astype(np.int64)
    et2 = np.concatenate([edge_type, edge_type]).astype(np.int64)
    ew2 = np.concatenate([edge_weights, edge_weights]).astype(f32)

    owner = tgt2 // nl
    tloc = tgt2 - owner * nl
    blkg = tloc // 128
    tin = (tloc - blkg * 128).astype(f32)
    halfid = np.digitize(blkg, bnds[1:-1])

    cnt_chr = np.bincount(
        (owner * H + halfid) * R + et2, minlength=NCORE * H * R
    ).reshape(NCORE, H, R)

    cnt_chrb = np.bincount(
        ((owner * H + halfid) * R + et2) * nblk + blkg,
        minlength=NCORE * H * R * nblk).reshape(NCORE, H, R, nblk)

    halves = []
    for h in range(H):
        b0, b1 = hb[h]
        for SL in SLC:
            n_iv = int(np.ceil(cnt_chrb[:, h, :, b0:b1] / SL)
                       .sum(axis=1).max())
            if n_iv <= 128:
                break
        else:
            raise AssertionError(f"no SL fits: {n_iv}")
        nch_r = np.ceil((cnt_chr[:, h].max(axis=0) + SL) / 128.0
                        ).astype(np.int64)
        cb = np.concatenate([[0], np.cumsum(nch_r)]).astype(np.int64)
        nch_h = int(cb[-1])
        groups = []
        for r in range(R):
            for g0 in range(0, int(nch_r[r]), WG):
                ng_ = int(min(WG, int(nch_r[r]) - g0))
                cf = int(cb[r] + g0)
                groups.append((128 * cf, cf, ng_, r))
        halves.append(dict(nch=nch_h, ep1=128 * nch_h, cb=cb, nch_r=nch_r,
                           groups=tuple(groups), b0=b0, b1=b1,
                           nbh=b1 - b0, SL=int(SL)))

    per_core = []
    for c in range(NCORE):
        dcore = {"wsb": wsb_h}
        xm = (x16[c * nl:(c + 1) * nl]
              * node_keep_mask[c * nl:(c + 1) * nl, None])
        xmt = np.zeros((128, nlp), f16)
        xmt[:, :nl] = xm.T.astype(f16)
        dcore["xmt"] = np.ascontiguousarray(xmt)
        for h in range(H):
            hs = halves[h]
            b0, nbh, SL = hs["b0"], hs["nbh"], hs["SL"]
            cbs, nch_r, nch_h, ep1 = (hs["cb"], hs["nch_r"], hs["nch"],
                                      hs["ep1"])
            sel = np.nonzero((owner == c) & (halfid == h))[0]
            et_s = et2[sel]
            blk_s = blkg[sel] - b0
            order = np.lexsort((blk_s, et_s))
            ranks = _ranks_within_group(et_s, order, R)
            rows = 128 * cbs[et_s] + ranks
            nch_of = nch_r[et_s]
            g = ranks // (128 * WG)
            ng_of = np.minimum(WG, nch_of - WG * g)
            om = ranks - g * 128 * WG
            e = om // ng_of
            jj = om - e * ng_of
            chunk = cbs[et_s] + WG * g + jj
            xcol = chunk * 128 + e
            XS = np.zeros((128 * nch_h, d), f16)
            XS[xcol] = (x16[src2[sel]] * ew2[sel][:, None]).astype(f16)
            dcore[f"xst{h}"] = np.ascontiguousarray(XS.T)

            edge_of_row = np.full(ep1, -1, np.int64)
            edge_of_row[rows] = sel
            cnt_rb = cnt_chrb[c, h, :, b0:hs["b1"]]
            run_start = np.zeros_like(cnt_rb)
            run_start[:, 1:] = np.cumsum(cnt_rb, axis=1)[:, :-1]
            # pad entries point at the tail pad rows (written, no edges)
            cidx = np.full((128, nbh), ep1 - SL, np.int32)
            for b in range(nbh):
                iv = []
                for r in range(R):
                    s = 128 * int(cbs[r]) + int(run_start[r, b])
                    ln = int(cnt_rb[r, b])
                    limit = 128 * int(cbs[r] + nch_r[r]) - SL
                    for off in range(0, ln, SL):
                        iv.append(min(s + off, limit))
                assert len(iv) <= 128, f"cover overflow {len(iv)}"
                cidx[:len(iv), b] = iv
            rowsm = cidx.astype(np.int64)[:, :, None] + np.arange(SL)
            evm = edge_of_row[rowsm]
            valid = evm >= 0
            evc = np.where(valid, evm, 0)
            use = (valid
                   & ((blkg[evc] - b0) == np.arange(nbh)[None, :, None])
                   & (halfid[evc] == h))
            assert int(use.sum()) == len(sel), (
                f"cover mismatch {int(use.sum())} vs {len(sel)}")
            tcol = np.where(use, tin[evc], -1.0).astype(f16)
            dcore[f"cidx{h}"] = np.ascontiguousarray(cidx)
            dcore[f"tcol{h}"] = np.ascontiguousarray(
                tcol.reshape(128, nbh * SL))
        per_core.append(dcore)

    cfg = dict(R=R, nlp=nlp, nblk=nblk,
               halves=tuple((hs["nch"], hs["ep1"], hs["b0"], hs["b1"],
                             hs["SL"], hs["groups"]) for hs in halves))
    return per_core, cfg


def build_program(cfg):
    R = cfg["R"]
    nlp = cfg["nlp"]
    SLmax = max(hv[4] for hv in cfg["halves"])

    nc = bacc.Bacc(None, target_bir_lowering=False, debug=False)
    wsb = nc.declare_dram_parameter("wsb", [128, (R + 1) * 128], F16,
                                    isOutput=False)
    xmt = nc.declare_dram_parameter("xmt", [128, nlp], F16, isOutput=False)
    hp = []
    for h, (nch_h, ep1, b0, b1, SL, groups) in enumerate(cfg["halves"]):
        xst = nc.declare_dram_parameter(f"xst{h}", [128, nch_h * 128], F16,
                                        isOutput=False)
        cidx = nc.declare_dram_parameter(f"cidx{h}", [128, b1 - b0], I32,
                                         isOutput=False)
        tcol = nc.declare_dram_parameter(f"tcol{h}", [128, (b1 - b0) * SL],
                                         F16, isOutput=False)
        md = nc.dram_tensor(f"md{h}", [ep1, 128], F16)
        hp.append((xst, cidx, tcol, md))
    outp = nc.declare_dram_parameter("out", [nlp, 128], F16, isOutput=True)

    colidx_d = nc.inline_tensor(
        np.tile(np.arange(128, dtype=np.float16), (128, SLmax)),
        name="colidx_c")

    with tile.TileContext(nc) as tc:
        with (
            tc.tile_pool(name="const", bufs=1) as constp,
            tc.tile_pool(name="rd", bufs=8) as rdp,
            tc.tile_pool(name="msb", bufs=4) as msbp,
            tc.tile_pool(name="p1ps", bufs=4, space="PSUM") as p1ps,
            tc.tile_pool(name="mg", bufs=10) as mgp,
            tc.tile_pool(name="tt", bufs=8) as ttp,
            tc.tile_pool(name="ob", bufs=4) as obp,
            tc.tile_pool(name="p2ps", bufs=4, space="PSUM") as p2ps,
        ):
            wsb_t = constp.tile([128, (R + 1) * 128], F16)
            nc.sync.dma_start(out=wsb_t[:], in_=wsb[:])
            xmt_t = constp.tile([128, nlp], F16)
            nc.sync.dma_start(out=xmt_t[:], in_=xmt[:])
            colidx = constp.tile([128, SLmax, 128], F16)
            nc.sync.dma_start(out=colidx[:], in_=colidx_d[:])
            cidx_ts, tcol_ts = [], []
            for h, (nch_h, ep1, b0, b1, SL, groups) in enumerate(
                    cfg["halves"]):
                nbh = b1 - b0
                ct = constp.tile([128, nbh], I32, name=f"cidx_t{h}")
                nc.sync.dma_start(out=ct[:], in_=hp[h][1][:])
                tc_ = constp.tile([128, nbh * SL], F16, name=f"tcol_t{h}")
                nc.sync.dma_start(out=tc_[:], in_=hp[h][2][:])
                cidx_ts.append(ct)
                tcol_ts.append(tc_)

            read_cache = [dict() for _ in range(H)]
            alt = [0]  # cast engine alternator
            alt_w = [0]  # md write engine alternator

            def _issue_read(h, bi):
                nch_h = cfg["halves"][h][0]
                nbat = (nch_h + KR - 1) // KR
                rc = read_cache[h]
                if bi >= nbat or bi in rc:
                    return
                w = min(KR, nch_h - bi * KR)
                rt = rdp.tile([128, KR * 128], F16, tag="rt")
                nc.sync.dma_start(
                    out=rt[:, :w * 128],
                    in_=hp[h][0][:, bi * KR * 128:(bi * KR + w) * 128])
                rc[bi] = rt

            def get_read(h, ci):
                bi = ci // KR
                rc = read_cache[h]
                for d in (0, 1, 2):  # prefetch two batches ahead
                    _issue_read(h, bi + d)
                for old in [k for k in rc if k < bi]:
                    del rc[old]
                return rc[bi]

            def emit_p1_group(h, gi):
                groups = cfg["halves"][h][5]
                md_d = hp[h][3]
                rowbase, cf, ng_, rel = groups[gi]
                msb = msbp.tile([128, WG * 128], F16, tag="msb")
                for s0 in range(0, ng_, PG):
                    sn = min(PG, ng_ - s0)
                    mp = p1ps.tile([128, PG * 128], F32, tag="mp")
                    for j in range(sn):
                        ci = cf + s0 + j
                        rt = get_read(h, ci)
                        off = (ci % KR) * 128
                        nc.tensor.matmul(
                            out=mp[:, j * 128:(j + 1) * 128],
                            lhsT=rt[:, off:off + 128],
                            rhs=wsb_t[:, rel * 128:(rel + 1) * 128],
                            start=True, stop=True)
                    if alt[0] % 2 == 0:
                        nc.vector.tensor_copy(
                            out=msb[:, s0 * 128:(s0 + sn) * 128],
                            in_=mp[:, :sn * 128])
                    else:
                        nc.scalar.copy(
                            out=msb[:, s0 * 128:(s0 + sn) * 128],
                            in_=mp[:, :sn * 128])
                    alt[0] += 1
                dst = md_d[rowbase:rowbase + 128 * ng_, :].rearrange(
                    "(e j) o -> e j o", j=ng_)
                weng = nc.scalar if alt_w[0] % 2 == 0 else nc.sync
                alt_w[0] += 1
                weng.dma_start(out=dst, in_=msb[:, :ng_ * 128])

            ob_state = [None, 0]

            def emit_p2_block(h, b):
                nch_h, ep1, b0, b1, SL, groups = cfg["halves"][h]
                md_d = hp[h][3]
                mg = mgp.tile([128, SLmax * 128], F16, tag="mg")
                nc.gpsimd.indirect_dma_start(
                    out=mg[:, :SL * 128], out_offset=None, in_=md_d[:, :],
                    in_offset=bass.IndirectOffsetOnAxis(
                        ap=cidx_ts[h][:, b:b + 1], axis=0))
                tt = ttp.tile([128, SLmax, 128], F16, tag="tt")
                nc.vector.tensor_tensor(
                    out=tt[:, :SL, :], in0=colidx[:, :SL, :],
                    in1=tcol_ts[h][:, b * SL:(b + 1) * SL].unsqueeze(2)
                    .to_broadcast([128, SL, 128]),
                    op=mybir.AluOpType.is_equal)
                ps = p2ps.tile([128, 128], F32, tag="acc")
                for j in range(SL):
                    nc.tensor.matmul(
                        out=ps[:], lhsT=tt[:, j, :],
                        rhs=mg[:, j * 128:(j + 1) * 128],
                        start=(j == 0), stop=False)
                gb = b0 + b
                nc.tensor.matmul(
                    out=ps[:], lhsT=xmt_t[:, gb * 128:(gb + 1) * 128],
                    rhs=wsb_t[:, R * 128:(R + 1) * 128],
                    start=False, stop=True)
                if ob_state[0] is None:
                    ob_state[0] = obp.tile([128, OB * 128], F16, tag="ob",
                                           name="obbig")
                    ob_state[1] = gb
                i = gb - ob_state[1]
                nc.scalar.copy(out=ob_state[0][:, i * 128:(i + 1) * 128],
                               in_=ps[:])
                if i == OB - 1 or b == b1 - b0 - 1:
                    nb = i + 1
                    gb0 = ob_state[1]
                    dst = outp[gb0 * 128:(gb0 + nb) * 128, :].rearrange(
                        "(b t) o -> t b o", b=nb)
                    nc.sync.dma_start(out=dst, in_=ob_state[0][:, :nb * 128])
                    ob_state[0] = None

            # schedule: p1(0); for h>=1: p2(h-1) interleaved with p1(h);
            # then p2(H-1)
            ngs = [len(cfg["halves"][h][5]) for h in range(H)]
            nbs = [cfg["halves"][h][3] - cfg["halves"][h][2]
                   for h in range(H)]
            for gi in range(ngs[0]):
                emit_p1_group(0, gi)
            for h in range(1, H):
                k = 0
                for b in range(nbs[h - 1]):
                    emit_p2_block(h - 1, b)
                    take = ((b + 1) * ngs[h]) // nbs[h - 1] \
                        - (b * ngs[h]) // nbs[h - 1]
                    for _ in range(take):
                        emit_p1_group(h, k)
                        k += 1
                while k < ngs[h]:
                    emit_p1_group(h, k)
                    k += 1
            for b in range(nbs[H - 1]):
                emit_p2_block(H - 1, b)

    nc.finalize()
    return nc


_PROGRAM_CACHE = {}


def _get_program(cfg):
    key = (cfg["R"], cfg["nlp"], cfg["nblk"], cfg["halves"])
    if key not in _PROGRAM_CACHE:
        _PROGRAM_CACHE[key] = build_program(cfg)
    return _PROGRAM_CACHE[key]


def kernel(x, node_keep_mask, source, target, edge_type, edge_weights,
           bases, relation_base_weights):
    per_core, cfg = host_prep(x, node_keep_mask, source, target, edge_type,
                              edge_weights, bases, relation_base_weights)
    nc = _get_program(cfg)
    res = run_bass_kernel_spmd(nc, per_core, list(range(NCORE)))
    n = x.shape[0]
    nl = n // NCORE
    out = np.empty((n, 128), np.float32)
    for c in range(NCORE):
        out[c * nl:(c + 1) * nl] = res.results[c]["out"][:nl].astype(
            np.float32)
    return out


# revision 23
# speedup vs baseline: 3.5849x; 1.0178x over previous
"""BasesDecomposition (R-GCN style) message passing kernel for Trainium2.

V4 strategy (8 NeuronCores, SPMD — one program, per-core data):
  - Nodes sharded by row: core c owns targets [c*NL, (c+1)*NL).
  - Edges symmetrized on host, partitioned by target-owner core, then by
    pipeline third (target-block range) and relation.
  - Host pre-gathers source features: XST[:, slot] = ew_e * x[src_e]
    (transposed, fp16, edge-weight folded in) so the device does no
    per-edge gathers and no transposes in phase 1.
  - Phase 1 (messages): per 128-edge relation-pure chunk, one fp16
    matmul XST_chunk.T @ W_r -> PSUM (4 chunks per PSUM bank, one
    fp32->fp16 copy each, alternating vector/scalar engines). Up to WG
    chunks share ONE grouped DMA write to the message buffer md. The md
    slot map row = rowbase_g + e*n_g + j keeps the write contiguous per
    partition while ranks (target-block-sorted within each relation)
    stay contiguous for phase-2 interval gathers.
  - Phase 2 (aggregate): per 128-target block, ONE indirect interval
    gather (128 intervals x SL rows) covering the block's per-relation
    runs; ONE broadcast is_equal builds all SL one-hot matrices; SL
    scatter matmuls accumulate out[t, o] += T_j.T @ M_j in PSUM, plus a
    self-loop matmul; fp16 copy + direct DMA to the output.
  - H pipeline stages: phase 2 of stage h-1 is interleaved with phase 1
    of stage h so DMA/gpsimd/PE/DVE work overlaps across stages.
"""

import numpy as np
import ml_dtypes

import concourse.bass as bass
import concourse.bacc as bacc
import concourse.tile as tile
import concourse.mybir as mybir
from concourse.bass_utils import run_bass_kernel_spmd

F8 = mybir.dt.float8e4
F16 = mybir.dt.float16
F32 = mybir.dt.float32
I32 = mybir.dt.int32
F8NP = ml_dtypes.float8_e4m3fn

NCORE = 8
H = 4            # pipeline stages (target-block ranges)
HW_ = (0.2, 0.3, 0.3, 0.2)  # stage size fractions (taper head/tail)
WG = 16          # chunks per md write group (slot-map group size)
PG = 4           # chunks per PSUM bank / cast
KR = 16          # chunks per XST read DMA
OB = 4           # output blocks per batched write
TB = 4           # phase-2 blocks per one-hot (tt) read DMA
SLC = (8, 10, 12, 16)  # md rows per cover interval in phase 2


def _ranks_within_group(keys, order, nbins):
    counts = np.bincount(keys, minlength=nbins)
    starts = np.concatenate([[0], np.cumsum(counts)[:-1]])
    r = np.empty(len(keys), np.int64)
    r[order] = np.arange(len(keys)) - starts[keys[order]]
    return r


def host_prep(x, node_keep_mask, source, target, edge_type, edge_weights,
              bases, relation_base_weights):
    n, d = x.shape
    assert d == 128 and n % NCORE == 0
    R = relation_base_weights.shape[0] - 1
    nl = n // NCORE
    nblk = (nl + 127) // 128
    nlp = nblk * 128
    # H block ranges, tapered so first/last stages are smaller
    cw = np.cumsum((0.0,) + HW_) / sum(HW_)
    bnds = [round(nblk * float(c)) for c in cw]
    hb = [(bnds[i], bnds[i + 1]) for i in range(H)]
    f16, f32 = np.float16, np.float32

    W = np.einsum("rb,bdo->rdo", relation_base_weights.astype(f32),
                  bases.astype(f32)).astype(f32)
    wsb_h = np.ascontiguousarray(
        W.transpose(1, 0, 2).reshape(d, (R + 1) * d)).astype(f16)

    x16 = x.astype(f16).astype(f32)  # quantize once, scale in f32
    src2 = np.concatenate([source, target]).astype(np.int64)
    tgt2 = np.concatenate([target, source]).astype(np.int64)
    et2 = np.concatenate([edge_type, edge_type]).astype(np.int64)
    ew2 = np.concatenate([edge_weights, edge_weights]).astype(f32)

    owner = tgt2 // nl
    tloc = tgt2 - owner * nl
    blkg = tloc // 128
    tin = (tloc - blkg * 128).astype(f32)
    halfid = np.digitize(blkg, bnds[1:-1])

    cnt_chr = np.bincount(
        (owner * H + halfid) * R + et2, minlength=NCORE * H * R
    ).reshape(NCORE, H, R)

    cnt_chrb = np.bincount(
        ((owner * H + halfid) * R + et2) * nblk + blkg,
        minlength=NCORE * H * R * nblk).reshape(NCORE, H, R, nblk)

    halves = []
    for h in range(H):
        b0, b1 = hb[h]
        for SL in SLC:
            n_iv = int(np.ceil(cnt_chrb[:, h, :, b0:b1] / SL)
                       .sum(axis=1).max())
            if n_iv <= 128:
                break
        else:
            raise AssertionError(f"no SL fits: {n_iv}")
        nch_r = np.ceil((cnt_chr[:, h].max(axis=0) + SL) / 128.0
                        ).astype(np.int64)
        cb = np.concatenate([[0], np.cumsum(nch_r)]).astype(np.int64)
        nch_h = int(cb[-1])
        groups = []
        for r in range(R):
            for g0 in range(0, int(nch_r[r]), WG):
                ng_ = int(min(WG, int(nch_r[r]) - g0))
                cf = int(cb[r] + g0)
                groups.append((128 * cf, cf, ng_, r))
        halves.append(dict(nch=nch_h, ep1=128 * nch_h, cb=cb, nch_r=nch_r,
                           groups=tuple(groups), b0=b0, b1=b1,
                           nbh=b1 - b0, SL=int(SL)))

    per_core = []
    for c in range(NCORE):
        dcore = {"wsb": wsb_h}
        xm = (x16[c * nl:(c + 1) * nl]
              * node_keep_mask[c * nl:(c + 1) * nl, None])
        xmt = np.zeros((128, nlp), f16)
        xmt[:, :nl] = xm.T.astype(f16)
        dcore["xmt"] = np.ascontiguousarray(xmt)
        for h in range(H):
            hs = halves[h]
            b0, nbh, SL = hs["b0"], hs["nbh"], hs["SL"]
            cbs, nch_r, nch_h, ep1 = (hs["cb"], hs["nch_r"], hs["nch"],
                                      hs["ep1"])
            sel = np.nonzero((owner == c) & (halfid == h))[0]
            et_s = et2[sel]
            blk_s = blkg[sel] - b0
            order = np.lexsort((blk_s, et_s))
            ranks = _ranks_within_group(et_s, order, R)
            rows = 128 * cbs[et_s] + ranks
            nch_of = nch_r[et_s]
            g = ranks // (128 * WG)
            ng_of = np.minimum(WG, nch_of - WG * g)
            om = ranks - g * 128 * WG
            e = om // ng_of
            jj = om - e * ng_of
            chunk = cbs[et_s] + WG * g + jj
            xcol = chunk * 128 + e
            XS = np.zeros((128 * nch_h, d), f16)
            XS[xcol] = (x16[src2[sel]] * ew2[sel][:, None]).astype(f16)
            dcore[f"xst{h}"] = np.ascontiguousarray(XS.T)

            edge_of_row = np.full(ep1, -1, np.int64)
            edge_of_row[rows] = sel
            cnt_rb = cnt_chrb[c, h, :, b0:hs["b1"]]
            run_start = np.zeros_like(cnt_rb)
            run_start[:, 1:] = np.cumsum(cnt_rb, axis=1)[:, :-1]
            # pad entries point at the tail pad rows (written, no edges)
            cidx = np.full((128, nbh), ep1 - SL, np.int32)
            for b in range(nbh):
                iv = []
                for r in range(R):
                    s = 128 * int(cbs[r]) + int(run_start[r, b])
                    ln = int(cnt_rb[r, b])
                    limit = 128 * int(cbs[r] + nch_r[r]) - SL
                    for off in range(0, ln, SL):
                        iv.append(min(s + off, limit))
                assert len(iv) <= 128, f"cover overflow {len(iv)}"
                cidx[:len(iv), b] = iv
            rowsm = cidx.astype(np.int64)[:, :, None] + np.arange(SL)
            evm = edge_of_row[rowsm]
            valid = evm >= 0
            evc = np.where(valid, evm, 0)
            use = (valid
                   & ((blkg[evc] - b0) == np.arange(nbh)[None, :, None])
                   & (halfid[evc] == h))
            assert int(use.sum()) == len(sel), (
                f"cover mismatch {int(use.sum())} vs {len(sel)}")
            tcol = np.where(use, tin[evc], -1.0)  # [128, nbh, SL]
            tt8 = (tcol[:, :, :, None] == np.arange(128, dtype=f32)
                   ).astype(F8NP)
            dcore[f"cidx{h}"] = np.ascontiguousarray(cidx)
            dcore[f"ttd{h}"] = np.ascontiguousarray(
                tt8.reshape(128, nbh * SL * 128))
        per_core.append(dcore)

    cfg = dict(R=R, nlp=nlp, nblk=nblk,
               halves=tuple((hs["nch"], hs["ep1"], hs["b0"], hs["b1"],
                             hs["SL"], hs["groups"]) for hs in halves))
    return per_core, cfg


def build_program(cfg):
    R = cfg["R"]
    nlp = cfg["nlp"]
    SLmax = max(hv[4] for hv in cfg["halves"])

    nc = bacc.Bacc(None, target_bir_lowering=False, debug=False)
    wsb = nc.declare_dram_parameter("wsb", [128, (R + 1) * 128], F16,
                                    isOutput=False)
    xmt = nc.declare_dram_parameter("xmt", [128, nlp], F16, isOutput=False)
    hp = []
    for h, (nch_h, ep1, b0, b1, SL, groups) in enumerate(cfg["halves"]):
        xst = nc.declare_dram_parameter(f"xst{h}", [128, nch_h * 128], F16,
                                        isOutput=False)
        cidx = nc.declare_dram_parameter(f"cidx{h}", [128, b1 - b0], I32,
                                         isOutput=False)
        ttd = nc.declare_dram_parameter(f"ttd{h}",
                                        [128, (b1 - b0) * SL * 128],
                                        F8, isOutput=False)
        md = nc.dram_tensor(f"md{h}", [ep1, 128], F16)
        hp.append((xst, cidx, ttd, md))
    outp = nc.declare_dram_parameter("out", [nlp, 128], F16, isOutput=True)

    with tile.TileContext(nc) as tc:
        with (
            tc.tile_pool(name="const", bufs=1) as constp,
            tc.tile_pool(name="rd", bufs=8) as rdp,
            tc.tile_pool(name="msb", bufs=4) as msbp,
            tc.tile_pool(name="p1ps", bufs=4, space="PSUM") as p1ps,
            tc.tile_pool(name="mg", bufs=10) as mgp,
            tc.tile_pool(name="tt", bufs=4) as ttp,
            tc.tile_pool(name="ob", bufs=4) as obp,
            tc.tile_pool(name="p2ps", bufs=4, space="PSUM") as p2ps,
        ):
            wsb_t = constp.tile([128, (R + 1) * 128], F16)
            nc.sync.dma_start(out=wsb_t[:], in_=wsb[:])
            xmt_t = constp.tile([128, nlp], F16)
            nc.sync.dma_start(out=xmt_t[:], in_=xmt[:])
            cidx_ts = []
            for h, (nch_h, ep1, b0, b1, SL, groups) in enumerate(
                    cfg["halves"]):
                nbh = b1 - b0
                ct = constp.tile([128, nbh], I32, name=f"cidx_t{h}")
                nc.sync.dma_start(out=ct[:], in_=hp[h][1][:])
                cidx_ts.append(ct)

            read_cache = [dict() for _ in range(H)]
            alt = [0]  # cast engine alternator
            alt_w = [0]  # md write engine alternator

            def _issue_read(h, bi):
                nch_h = cfg["halves"][h][0]
                nbat = (nch_h + KR - 1) // KR
                rc = read_cache[h]
                if bi >= nbat or bi in rc:
                    return
                w = min(KR, nch_h - bi * KR)
                rt = rdp.tile([128, KR * 128], F16, tag="rt")
                nc.sync.dma_start(
                    out=rt[:, :w * 128],
                    in_=hp[h][0][:, bi * KR * 128:(bi * KR + w) * 128])
                rc[bi] = rt

            def get_read(h, ci):
                bi = ci // KR
                rc = read_cache[h]
                for d in (0, 1, 2):  # prefetch two batches ahead
                    _issue_read(h, bi + d)
                for old in [k for k in rc if k < bi]:
                    del rc[old]
                return rc[bi]

            def emit_p1_group(h, gi):
                groups = cfg["halves"][h][5]
                md_d = hp[h][3]
                rowbase, cf, ng_, rel = groups[gi]
                msb = msbp.tile([128, WG * 128], F16, tag="msb")
                for s0 in range(0, ng_, PG):
                    sn = min(PG, ng_ - s0)
                    mp = p1ps.tile([128, PG * 128], F32, tag="mp")
                    for j in range(sn):
                        ci = cf + s0 + j
                        rt = get_read(h, ci)
                        off = (ci % KR) * 128
                        nc.tensor.matmul(
                            out=mp[:, j * 128:(j + 1) * 128],
                            lhsT=rt[:, off:off + 128],
                            rhs=wsb_t[:, rel * 128:(rel + 1) * 128],
                            start=True, stop=True)
                    if alt[0] % 2 == 0:
                        nc.vector.tensor_copy(
                            out=msb[:, s0 * 128:(s0 + sn) * 128],
                            in_=mp[:, :sn * 128])
                    else:
                        nc.scalar.copy(
                            out=msb[:, s0 * 128:(s0 + sn) * 128],
                            in_=mp[:, :sn * 128])
                    alt[0] += 1
                dst = md_d[rowbase:rowbase + 128 * ng_, :].rearrange(
                    "(e j) o -> e j o", j=ng_)
                weng = nc.scalar if alt_w[0] % 2 == 0 else nc.sync
                alt_w[0] += 1
                weng.dma_start(out=dst, in_=msb[:, :ng_ * 128])

            ob_state = [None, 0]
            tt_cache = [dict() for _ in range(H)]

            def get_tt(h, b):
                nbh = cfg["halves"][h][3] - cfg["halves"][h][2]
                SL = cfg["halves"][h][4]
                qi = b // TB
                rc = tt_cache[h]
                if qi not in rc:
                    w = min(TB, nbh - qi * TB)
                    ttq = ttp.tile([128, TB * SLmax * 128], F8, tag="ttq",
                                   name="ttq")
                    nc.sync.dma_start(
                        out=ttq[:, :w * SL * 128],
                        in_=hp[h][2][:, qi * TB * SL * 128:
                                     (qi * TB + w) * SL * 128])
                    for old in [k for k in rc if k < qi]:
                        del rc[old]
                    rc[qi] = ttq
                return rc[qi]

            def emit_p2_block(h, b):
                nch_h, ep1, b0, b1, SL, groups = cfg["halves"][h]
                md_d = hp[h][3]
                mg = mgp.tile([128, SLmax * 128], F16, tag="mg")
                nc.gpsimd.indirect_dma_start(
                    out=mg[:, :SL * 128], out_offset=None, in_=md_d[:, :],
                    in_offset=bass.IndirectOffsetOnAxis(
                        ap=cidx_ts[h][:, b:b + 1], axis=0))
                ttq = get_tt(h, b)
                toff = (b % TB) * SL * 128
                ps = p2ps.tile([128, 128], F32, tag="acc")
                for j in range(SL):
                    nc.tensor.matmul(
                        out=ps[:],
                        lhsT=ttq[:, toff + j * 128:toff + (j + 1) * 128],
                        rhs=mg[:, j * 128:(j + 1) * 128],
                        start=(j == 0), stop=False)
                gb = b0 + b
                nc.tensor.matmul(
                    out=ps[:], lhsT=xmt_t[:, gb * 128:(gb + 1) * 128],
                    rhs=wsb_t[:, R * 128:(R + 1) * 128],
                    start=False, stop=True)
                if ob_state[0] is None:
                    ob_state[0] = obp.tile([128, OB * 128], F16, tag="ob",
                                           name="obbig")
                    ob_state[1] = gb
                i = gb - ob_state[1]
                nc.scalar.copy(out=ob_state[0][:, i * 128:(i + 1) * 128],
                               in_=ps[:])
                if i == OB - 1 or b == b1 - b0 - 1:
                    nb = i + 1
                    gb0 = ob_state[1]
                    dst = outp[gb0 * 128:(gb0 + nb) * 128, :].rearrange(
                        "(b t) o -> t b o", b=nb)
                    nc.sync.dma_start(out=dst, in_=ob_state[0][:, :nb * 128])
                    ob_state[0] = None

            # schedule: p1(0); for h>=1: p2(h-1) interleaved with p1(h);
            # then p2(H-1)
            ngs = [len(cfg["halves"][h][5]) for h in range(H)]
            nbs = [cfg["halves"][h][3] - cfg["halves"][h][2]
                   for h in range(H)]
            for gi in range(ngs[0]):
                emit_p1_group(0, gi)
            for h in range(1, H):
                k = 0
                for b in range(nbs[h - 1]):
                    emit_p2_block(h - 1, b)
                    take = ((b + 1) * ngs[h]) // nbs[h - 1] \
                        - (b * ngs[h]) // nbs[h - 1]
                    for _ in range(take):
                        emit_p1_group(h, k)
                        k += 1
                while k < ngs[h]:
                    emit_p1_group(h, k)
                    k += 1
            for b in range(nbs[H - 1]):
                emit_p2_block(H - 1, b)

    nc.finalize()
    return nc


_PROGRAM_CACHE = {}


def _get_program(cfg):
    key = (cfg["R"], cfg["nlp"], cfg["nblk"], cfg["halves"])
    if key not in _PROGRAM_CACHE:
        _PROGRAM_CACHE[key] = build_program(cfg)
    return _PROGRAM_CACHE[key]


def kernel(x, node_keep_mask, source, target, edge_type, edge_weights,
           bases, relation_base_weights):
    per_core, cfg = host_prep(x, node_keep_mask, source, target, edge_type,
                              edge_weights, bases, relation_base_weights)
    nc = _get_program(cfg)
    res = run_bass_kernel_spmd(nc, per_core, list(range(NCORE)))
    n = x.shape[0]
    nl = n // NCORE
    out = np.empty((n, 128), np.float32)
    for c in range(NCORE):
        out[c * nl:(c + 1) * nl] = res.results[c]["out"][:nl].astype(
            np.float32)
    return out


# revision 32
# speedup vs baseline: 3.8448x; 1.0725x over previous
"""BasesDecomposition (R-GCN style) message passing kernel for Trainium2.

V4 strategy (8 NeuronCores, SPMD — one program, per-core data):
  - Nodes sharded by row: core c owns targets [c*NL, (c+1)*NL).
  - Edges symmetrized on host, partitioned by target-owner core, then by
    pipeline third (target-block range) and relation.
  - Host pre-gathers source features: XST[:, slot] = ew_e * x[src_e]
    (transposed, fp16, edge-weight folded in) so the device does no
    per-edge gathers and no transposes in phase 1.
  - Phase 1 (messages): per 128-edge relation-pure chunk, one fp16
    matmul XST_chunk.T @ W_r -> PSUM (4 chunks per PSUM bank, one
    fp32->fp16 copy each, alternating vector/scalar engines). Up to WG
    chunks share ONE grouped DMA write to the message buffer md. The md
    slot map row = rowbase_g + e*n_g + j keeps the write contiguous per
    partition while ranks (target-block-sorted within each relation)
    stay contiguous for phase-2 interval gathers.
  - Phase 2 (aggregate): per 128-target block, ONE indirect interval
    gather (128 intervals x SL rows) covering the block's per-relation
    runs; ONE broadcast is_equal builds all SL one-hot matrices; SL
    scatter matmuls accumulate out[t, o] += T_j.T @ M_j in PSUM, plus a
    self-loop matmul; fp16 copy + direct DMA to the output.
  - H pipeline stages: phase 2 of stage h-1 is interleaved with phase 1
    of stage h so DMA/gpsimd/PE/DVE work overlaps across stages.
"""

import numpy as np
import ml_dtypes

import concourse.bass as bass
import concourse.bacc as bacc
import concourse.tile as tile
import concourse.mybir as mybir
from concourse.bass_utils import run_bass_kernel_spmd

F8 = mybir.dt.float8e4
F16 = mybir.dt.float16
F32 = mybir.dt.float32
I32 = mybir.dt.int32
F8NP = ml_dtypes.float8_e4m3fn

NCORE = 8
H = 4            # pipeline stages (target-block ranges)
HW_ = (0.2, 0.3, 0.3, 0.2)  # stage size fractions (taper head/tail)
WG = 16          # chunks per md write group (slot-map group size)
PG = 4           # chunks per PSUM bank / cast
KR = 16          # chunks per XST read DMA
OB = 4           # output blocks per batched write
SLC = (6, 7, 8, 10, 12, 16)  # per-block md rows per cover interval


def _ranks_within_group(keys, order, nbins):
    counts = np.bincount(keys, minlength=nbins)
    starts = np.concatenate([[0], np.cumsum(counts)[:-1]])
    r = np.empty(len(keys), np.int64)
    r[order] = np.arange(len(keys)) - starts[keys[order]]
    return r


def host_prep(x, node_keep_mask, source, target, edge_type, edge_weights,
              bases, relation_base_weights):
    n, d = x.shape
    assert d == 128 and n % NCORE == 0
    R = relation_base_weights.shape[0] - 1
    nl = n // NCORE
    nblk = (nl + 127) // 128
    nlp = nblk * 128
    # H block ranges, tapered so first/last stages are smaller
    cw = np.cumsum((0.0,) + HW_) / sum(HW_)
    bnds = [round(nblk * float(c)) for c in cw]
    hb = [(bnds[i], bnds[i + 1]) for i in range(H)]
    f16, f32 = np.float16, np.float32

    W = np.einsum("rb,bdo->rdo", relation_base_weights.astype(f32),
                  bases.astype(f32)).astype(f32)
    wsb_h = np.ascontiguousarray(
        W.transpose(1, 0, 2).reshape(d, (R + 1) * d)).astype(f16)

    x16 = x.astype(f16).astype(f32)  # quantize once, scale in f32
    src2 = np.concatenate([source, target]).astype(np.int64)
    tgt2 = np.concatenate([target, source]).astype(np.int64)
    et2 = np.concatenate([edge_type, edge_type]).astype(np.int64)
    ew2 = np.concatenate([edge_weights, edge_weights]).astype(f32)

    owner = tgt2 // nl
    tloc = tgt2 - owner * nl
    blkg = tloc // 128
    tin = (tloc - blkg * 128).astype(f32)
    halfid = np.digitize(blkg, bnds[1:-1])

    cnt_chr = np.bincount(
        (owner * H + halfid) * R + et2, minlength=NCORE * H * R
    ).reshape(NCORE, H, R)

    cnt_chrb = np.bincount(
        ((owner * H + halfid) * R + et2) * nblk + blkg,
        minlength=NCORE * H * R * nblk).reshape(NCORE, H, R, nblk)

    halves = []
    for h in range(H):
        b0, b1 = hb[h]
        # per-block minimal interval stride (shared across cores)
        SLs = []
        for b in range(b0, b1):
            for SL in SLC:
                n_iv = int(np.ceil(cnt_chrb[:, h, :, b] / SL)
                           .sum(axis=1).max())
                if n_iv <= 128:
                    break
            else:
                raise AssertionError(f"no SL fits: {n_iv}")
            SLs.append(int(SL))
        SLmax_h = max(SLs)
        toffs = [0]
        for s in SLs:
            toffs.append(toffs[-1] + s)
        nch_r = np.ceil((cnt_chr[:, h].max(axis=0) + SLmax_h) / 128.0
                        ).astype(np.int64)
        cb = np.concatenate([[0], np.cumsum(nch_r)]).astype(np.int64)
        nch_h = int(cb[-1])
        groups = []
        for r in range(R):
            for g0 in range(0, int(nch_r[r]), WG):
                ng_ = int(min(WG, int(nch_r[r]) - g0))
                cf = int(cb[r] + g0)
                groups.append((128 * cf, cf, ng_, r))
        halves.append(dict(nch=nch_h, ep1=128 * nch_h, cb=cb, nch_r=nch_r,
                           groups=tuple(groups), b0=b0, b1=b1,
                           nbh=b1 - b0, SLs=tuple(SLs),
                           toffs=tuple(toffs)))

    per_core = []
    for c in range(NCORE):
        dcore = {"wsb": wsb_h}
        xm = (x16[c * nl:(c + 1) * nl]
              * node_keep_mask[c * nl:(c + 1) * nl, None])
        xmt = np.zeros((128, nlp), f16)
        xmt[:, :nl] = xm.T.astype(f16)
        dcore["xmt"] = np.ascontiguousarray(xmt)
        for h in range(H):
            hs = halves[h]
            b0, nbh = hs["b0"], hs["nbh"]
            SLs, toffs = hs["SLs"], hs["toffs"]
            cbs, nch_r, nch_h, ep1 = (hs["cb"], hs["nch_r"], hs["nch"],
                                      hs["ep1"])
            sel = np.nonzero((owner == c) & (halfid == h))[0]
            et_s = et2[sel]
            blk_s = blkg[sel] - b0
            order = np.lexsort((blk_s, et_s))
            ranks = _ranks_within_group(et_s, order, R)
            rows = 128 * cbs[et_s] + ranks
            nch_of = nch_r[et_s]
            g = ranks // (128 * WG)
            ng_of = np.minimum(WG, nch_of - WG * g)
            om = ranks - g * 128 * WG
            e = om // ng_of
            jj = om - e * ng_of
            chunk = cbs[et_s] + WG * g + jj
            xcol = chunk * 128 + e
            XS = np.zeros((128 * nch_h, d), f16)
            XS[xcol] = (x16[src2[sel]] * ew2[sel][:, None]).astype(f16)
            dcore[f"xst{h}"] = np.ascontiguousarray(XS.T)

            edge_of_row = np.full(ep1, -1, np.int64)
            edge_of_row[rows] = sel
            cnt_rb = cnt_chrb[c, h, :, b0:hs["b1"]]
            run_start = np.zeros_like(cnt_rb)
            run_start[:, 1:] = np.cumsum(cnt_rb, axis=1)[:, :-1]
            cidx = np.zeros((128, nbh), np.int32)
            tcol = np.full((128, toffs[-1]), -1.0, f32)
            nuse = 0
            for b in range(nbh):
                SL = SLs[b]
                # pad entries point at tail pad rows (written, no edges)
                cidx[:, b] = ep1 - SL
                iv = []
                for r in range(R):
                    s = 128 * int(cbs[r]) + int(run_start[r, b])
                    ln = int(cnt_rb[r, b])
                    limit = 128 * int(cbs[r] + nch_r[r]) - SL
                    for off in range(0, ln, SL):
                        iv.append(min(s + off, limit))
                assert len(iv) <= 128, f"cover overflow {len(iv)}"
                cidx[:len(iv), b] = iv
                rowsm = cidx[:, b].astype(np.int64)[:, None] + np.arange(SL)
                evm = edge_of_row[rowsm]
                valid = evm >= 0
                evc = np.where(valid, evm, 0)
                use = (valid & (blkg[evc] - b0 == b) & (halfid[evc] == h))
                nuse += int(use.sum())
                tcol[:, toffs[b]:toffs[b + 1]] = np.where(
                    use, tin[evc], -1.0)
            assert nuse == len(sel), f"cover mismatch {nuse} vs {len(sel)}"
            dcore[f"cidx{h}"] = np.ascontiguousarray(cidx)
            dcore[f"tcol{h}"] = np.ascontiguousarray(tcol.astype(f16))
        per_core.append(dcore)

    cfg = dict(R=R, nlp=nlp, nblk=nblk,
               halves=tuple((hs["nch"], hs["ep1"], hs["b0"], hs["b1"],
                             hs["SLs"], hs["toffs"], hs["groups"])
                            for hs in halves))
    return per_core, cfg


def build_program(cfg):
    R = cfg["R"]
    nlp = cfg["nlp"]
    SLmax = max(max(hv[4]) for hv in cfg["halves"])

    nc = bacc.Bacc(None, target_bir_lowering=False, debug=False)
    wsb = nc.declare_dram_parameter("wsb", [128, (R + 1) * 128], F16,
                                    isOutput=False)
    xmt = nc.declare_dram_parameter("xmt", [128, nlp], F16, isOutput=False)
    hp = []
    for h, (nch_h, ep1, b0, b1, SLs, toffs, groups) in enumerate(
            cfg["halves"]):
        xst = nc.declare_dram_parameter(f"xst{h}", [128, nch_h * 128], F16,
                                        isOutput=False)
        cidx = nc.declare_dram_parameter(f"cidx{h}", [128, b1 - b0], I32,
                                         isOutput=False)
        tcol = nc.declare_dram_parameter(f"tcol{h}", [128, toffs[-1]], F16,
                                         isOutput=False)
        md = nc.dram_tensor(f"md{h}", [ep1, 128], F16)
        hp.append((xst, cidx, tcol, md))
    outp = nc.declare_dram_parameter("out", [nlp, 128], F16, isOutput=True)

    colidx_d = nc.inline_tensor(
        np.tile(np.arange(128, dtype=np.float16), (128, SLmax)),
        name="colidx_c")

    with tile.TileContext(nc) as tc:
        with (
            tc.tile_pool(name="const", bufs=1) as constp,
            tc.tile_pool(name="rd", bufs=8) as rdp,
            tc.tile_pool(name="msb", bufs=4) as msbp,
            tc.tile_pool(name="p1ps", bufs=4, space="PSUM") as p1ps,
            tc.tile_pool(name="mg", bufs=10) as mgp,
            tc.tile_pool(name="tt", bufs=6) as ttp,
            tc.tile_pool(name="ob", bufs=4) as obp,
            tc.tile_pool(name="p2ps", bufs=4, space="PSUM") as p2ps,
        ):
            wsb_t = constp.tile([128, (R + 1) * 128], F16)
            nc.sync.dma_start(out=wsb_t[:], in_=wsb[:])
            xmt_t = constp.tile([128, nlp], F16)
            nc.sync.dma_start(out=xmt_t[:], in_=xmt[:])
            colidx = constp.tile([128, SLmax, 128], F16)
            nc.sync.dma_start(out=colidx[:], in_=colidx_d[:])
            cidx_ts, tcol_ts = [], []
            for h, (nch_h, ep1, b0, b1, SLs, toffs, groups) in enumerate(
                    cfg["halves"]):
                nbh = b1 - b0
                ct = constp.tile([128, nbh], I32, name=f"cidx_t{h}")
                nc.sync.dma_start(out=ct[:], in_=hp[h][1][:])
                cidx_ts.append(ct)
                tc_ = constp.tile([128, toffs[-1]], F16, name=f"tcol_t{h}")
                nc.sync.dma_start(out=tc_[:], in_=hp[h][2][:])
                tcol_ts.append(tc_)

            read_cache = [dict() for _ in range(H)]
            alt = [0]  # cast engine alternator
            alt_w = [0]  # md write engine alternator

            def _issue_read(h, bi):
                nch_h = cfg["halves"][h][0]
                nbat = (nch_h + KR - 1) // KR
                rc = read_cache[h]
                if bi >= nbat or bi in rc:
                    return
                w = min(KR, nch_h - bi * KR)
                rt = rdp.tile([128, KR * 128], F16, tag="rt")
                nc.sync.dma_start(
                    out=rt[:, :w * 128],
                    in_=hp[h][0][:, bi * KR * 128:(bi * KR + w) * 128])
                rc[bi] = rt

            def get_read(h, ci):
                bi = ci // KR
                rc = read_cache[h]
                for d in (0, 1, 2):  # prefetch two batches ahead
                    _issue_read(h, bi + d)
                for old in [k for k in rc if k < bi]:
                    del rc[old]
                return rc[bi]

            def emit_p1_group(h, gi):
                groups = cfg["halves"][h][6]
                md_d = hp[h][3]
                rowbase, cf, ng_, rel = groups[gi]
                msb = msbp.tile([128, WG * 128], F16, tag="msb")
                for s0 in range(0, ng_, PG):
                    sn = min(PG, ng_ - s0)
                    mp = p1ps.tile([128, PG * 128], F32, tag="mp")
                    for j in range(sn):
                        ci = cf + s0 + j
                        rt = get_read(h, ci)
                        off = (ci % KR) * 128
                        nc.tensor.matmul(
                            out=mp[:, j * 128:(j + 1) * 128],
                            lhsT=rt[:, off:off + 128],
                            rhs=wsb_t[:, rel * 128:(rel + 1) * 128],
                            start=True, stop=True)
                    if alt[0] % 2 == 0:
                        nc.vector.tensor_copy(
                            out=msb[:, s0 * 128:(s0 + sn) * 128],
                            in_=mp[:, :sn * 128])
                    else:
                        nc.scalar.copy(
                            out=msb[:, s0 * 128:(s0 + sn) * 128],
                            in_=mp[:, :sn * 128])
                    alt[0] += 1
                dst = md_d[rowbase:rowbase + 128 * ng_, :].rearrange(
                    "(e j) o -> e j o", j=ng_)
                weng = nc.scalar if alt_w[0] % 2 == 0 else nc.sync
                alt_w[0] += 1
                weng.dma_start(out=dst, in_=msb[:, :ng_ * 128])

            ob_state = [None, 0]

            def emit_p2_block(h, b):
                nch_h, ep1, b0, b1, SLs, toffs, groups = cfg["halves"][h]
                SL = SLs[b]
                md_d = hp[h][3]
                mg = mgp.tile([128, SLmax * 128], F16, tag="mg")
                nc.gpsimd.indirect_dma_start(
                    out=mg[:, :SL * 128], out_offset=None, in_=md_d[:, :],
                    in_offset=bass.IndirectOffsetOnAxis(
                        ap=cidx_ts[h][:, b:b + 1], axis=0))
                tt = ttp.tile([128, SLmax, 128], F16, tag="tt")
                nc.vector.tensor_tensor(
                    out=tt[:, :SL, :], in0=colidx[:, :SL, :],
                    in1=tcol_ts[h][:, toffs[b]:toffs[b + 1]].unsqueeze(2)
                    .to_broadcast([128, SL, 128]),
                    op=mybir.AluOpType.is_equal)
                ps = p2ps.tile([128, 128], F32, tag="acc")
                for j in range(SL):
                    nc.tensor.matmul(
                        out=ps[:], lhsT=tt[:, j, :],
                        rhs=mg[:, j * 128:(j + 1) * 128],
                        start=(j == 0), stop=False)
                gb = b0 + b
                nc.tensor.matmul(
                    out=ps[:], lhsT=xmt_t[:, gb * 128:(gb + 1) * 128],
                    rhs=wsb_t[:, R * 128:(R + 1) * 128],
                    start=False, stop=True)
                if ob_state[0] is None:
                    ob_state[0] = obp.tile([128, OB * 128], F16, tag="ob",
                                           name="obbig")
                    ob_state[1] = gb
                i = gb - ob_state[1]
                nc.scalar.copy(out=ob_state[0][:, i * 128:(i + 1) * 128],
                               in_=ps[:])
                if i == OB - 1 or b == b1 - b0 - 1:
                    nb = i + 1
                    gb0 = ob_state[1]
                    dst = outp[gb0 * 128:(gb0 + nb) * 128, :].rearrange(
                        "(b t) o -> t b o", b=nb)
                    nc.sync.dma_start(out=dst, in_=ob_state[0][:, :nb * 128])
                    ob_state[0] = None

            # schedule: p1(0); for h>=1: p2(h-1) interleaved with p1(h);
            # then p2(H-1)
            ngs = [len(cfg["halves"][h][6]) for h in range(H)]
            nbs = [cfg["halves"][h][3] - cfg["halves"][h][2]
                   for h in range(H)]
            for gi in range(ngs[0]):
                emit_p1_group(0, gi)
            for h in range(1, H):
                k = 0
                for b in range(nbs[h - 1]):
                    emit_p2_block(h - 1, b)
                    take = ((b + 1) * ngs[h]) // nbs[h - 1] \
                        - (b * ngs[h]) // nbs[h - 1]
                    for _ in range(take):
                        emit_p1_group(h, k)
                        k += 1
                while k < ngs[h]:
                    emit_p1_group(h, k)
                    k += 1
            for b in range(nbs[H - 1]):
                emit_p2_block(H - 1, b)

    nc.finalize()
    return nc


_PROGRAM_CACHE = {}


def _get_program(cfg):
    key = (cfg["R"], cfg["nlp"], cfg["nblk"], cfg["halves"])
    if key not in _PROGRAM_CACHE:
        _PROGRAM_CACHE[key] = build_program(cfg)
    return _PROGRAM_CACHE[key]


def kernel(x, node_keep_mask, source, target, edge_type, edge_weights,
           bases, relation_base_weights):
    per_core, cfg = host_prep(x, node_keep_mask, source, target, edge_type,
                              edge_weights, bases, relation_base_weights)
    nc = _get_program(cfg)
    res = run_bass_kernel_spmd(nc, per_core, list(range(NCORE)))
    n = x.shape[0]
    nl = n // NCORE
    out = np.empty((n, 128), np.float32)
    for c in range(NCORE):
        out[c * nl:(c + 1) * nl] = res.results[c]["out"][:nl].astype(
            np.float32)
    return out
